# revision 30
# baseline (speedup 1.0000x reference)
"""GCN encoder kernel for Trainium2 (8 NeuronCores).

Strategy (graph/data parallel, per sharding hint):
  - Nodes sharded by destination range across 8 cores (2500 -> padded 2560/core).
  - Host precomputes GCN symmetric normalization and a per-destination-group
    schedule: non-self edges are deduplicated by source per 128-dst group and
    padded with trailing -1 (skipped by the gather ucode).  The segment-sum
    becomes dense matmuls with tiny scatter matrices S[src_slot, dst] holding
    the summed edge norms; self-loops are applied as a diagonal matmul against
    the locally-kept z.
  - Per layer: AllGather z (bf16) across cores, dma_gather the unique source
    rows, TensorE matmuls accumulate messages into PSUM per dst group (with the
    per-layer bias folded in as a rank-1 matmul), then GELU + LayerNorm
    (rsqrt via Newton on VectorE; no ACT table switches) + residual blend.
"""

import sys
from dataclasses import dataclass

sys.path.insert(0, "/opt/trn_rl_repo")

import numpy as np
import ml_dtypes

import concourse.bass as bass
import concourse.tile as tile
from concourse import bacc, mybir
from concourse import bass_utils
from concourse.masks import make_identity

BF16 = ml_dtypes.bfloat16
ALPHA = 0.1
EPS = 1e-5
IN_DIM = 128
H = 256
NCORES = 8
ACT_FN = mybir.ActivationFunctionType.Gelu_apprx_tanh
QUAKE_MAGIC = 0x5F3759DF


@dataclass(frozen=True)
class Cfg:
    n: int = 20000
    layers: int = 6

    @property
    def percore(self):
        return self.n // NCORES

    @property
    def groups(self):
        return (self.percore + 127) // 128

    @property
    def padn(self):
        return self.groups * 128


DEFAULT_CFG = Cfg()
_cache = {}


def _preprocess(cfg, x, edge_index, W_in, b_in, g_in, beta_in, Wc, bc, gc,
                betac):
    """Host-side graph preprocessing -> per-core input maps."""
    N, G, PERCORE, PADN, L = (cfg.n, cfg.groups, cfg.percore, cfg.padn,
                              cfg.layers)
    x = np.asarray(x, dtype=np.float32)
    ei = np.asarray(edge_index).astype(np.int64)
    W_in = np.asarray(W_in, dtype=np.float32)
    b_in = np.asarray(b_in, dtype=np.float32)
    g_in = np.asarray(g_in, dtype=np.float32)
    beta_in = np.asarray(beta_in, dtype=np.float32)
    Wc = np.asarray(Wc, dtype=np.float32)
    bc = np.asarray(bc, dtype=np.float32)
    gc = np.asarray(gc, dtype=np.float32)
    betac = np.asarray(betac, dtype=np.float32)

    loop = np.arange(N, dtype=np.int64)
    col_all = np.concatenate([ei[1], loop])   # dst (for degree)
    deg = np.bincount(col_all, minlength=N).astype(np.float32)
    dinv = np.where(deg > 0, 1.0 / np.sqrt(deg), 0.0).astype(np.float32)

    # non-self edges (self loops handled by the diagonal matmul)
    row = ei[0]
    col = ei[1]
    norm = (dinv[row] * dinv[col]).astype(np.float32)
    pad_src = (row // PERCORE) * PADN + (row % PERCORE)

    core_of = col // PERCORE
    dloc_all = col % PERCORE
    grp_all = dloc_all // 128
    dst_all = dloc_all % 128

    per_core = []
    maxU = 1
    for c in range(NCORES):
        m = core_of == c
        g_e = grp_all[m]
        d_e = dst_all[m]
        s_e = pad_src[m]
        n_e = norm[m]
        # dedup sources within each group
        key = g_e * (NCORES * PADN) + s_e
        ukey, slot_of_edge = np.unique(key, return_inverse=True)
        u_grp = ukey // (NCORES * PADN)
        u_src = ukey % (NCORES * PADN)
        counts = np.bincount(u_grp, minlength=G)
        maxU = max(maxU, counts.max())
        per_core.append((g_e, d_e, s_e, n_e, slot_of_edge, u_grp, u_src,
                         counts))

    C = int((maxU + 127) // 128)
    SLOTS = C * 128
    S16 = SLOTS // 16

    in_maps = []
    for c in range(NCORES):
        g_e, d_e, s_e, n_e, slot_of_edge, u_grp, u_src, counts = per_core[c]
        starts = np.zeros(G + 1, dtype=np.int64)
        np.cumsum(counts, out=starts[1:])
        # slot of each unique (group, src) within its group
        u_slot = np.arange(len(u_grp)) - starts[u_grp]

        # Pads are dummy index 0 (gathered but weighted 0 in S): groups are
        # batched 4-per-gather-instruction, so pads sit interior where the
        # ucode's trailing -1 trim cannot apply.
        idx_l = np.zeros((G, SLOTS), dtype=np.int16)
        idx_l[u_grp, u_slot] = u_src.astype(np.int16)

        S_l = np.zeros((G, SLOTS, 128), dtype=np.float32)
        np.add.at(S_l, (g_e, u_slot[slot_of_edge], d_e), n_e)

        # idx layout: logical slot i -> partition 16*stripe + i%16, col i//16
        idx_rs = idx_l.reshape(G, S16, 16).transpose(2, 0, 1)   # [16, G, S16]
        gidx = np.tile(idx_rs, (8, 1, 1)).astype(np.int16)      # [128, G, S16]

        # S layout: [G, 128(slot%128), C(chunk), 128(dst)]
        S_arr = S_l.reshape(G, C, 128, 128).transpose(0, 2, 1, 3).astype(BF16)

        # self-loop diagonal: D[g, p, p] = dinv[node]^2
        nodes = np.arange(PERCORE) + c * PERCORE
        d2 = np.zeros(PADN, dtype=np.float32)
        d2[:PERCORE] = dinv[nodes] ** 2
        D_arr = np.zeros((G, 128, 128), dtype=np.float32)
        pi = np.arange(128)
        for g in range(G):
            D_arr[g, pi, pi] = d2[g * 128:(g + 1) * 128]
        D_arr = D_arr.astype(BF16)

        xs = x[c * PERCORE:(c + 1) * PERCORE]
        xT = np.zeros((IN_DIM, PADN), dtype=BF16)
        xT[:, :PERCORE] = xs.T.astype(BF16)

        in_maps.append({
            "xT": np.ascontiguousarray(xT),
            "gidx": np.ascontiguousarray(gidx),
            "S": np.ascontiguousarray(S_arr),
            "D": np.ascontiguousarray(D_arr),
        })

    Wc_bf = np.ascontiguousarray(Wc.reshape(L, 2, 128, H).astype(BF16))
    W_in_bf = W_in.astype(BF16)
    biases = np.concatenate([b_in[None, :], bc], axis=0).astype(BF16)
    ln = np.zeros((2 + 2 * L, H), dtype=np.float32)
    ln[0] = g_in
    ln[1] = beta_in
    ln[2:2 + L] = (1.0 - ALPHA) * gc
    ln[2 + L:2 + 2 * L] = (1.0 - ALPHA) * betac
    for m in in_maps:
        m["Wc"] = Wc_bf
        m["W_in"] = W_in_bf
        m["biases"] = biases
        m["ln"] = ln

    return in_maps, C


def _build(cfg, C):
    """Build the Bass program (shared by all 8 cores)."""
    G, PADN, L = cfg.groups, cfg.padn, cfg.layers
    SLOTS = C * 128
    S16 = SLOTS // 16
    f32 = mybir.dt.float32
    i32 = mybir.dt.int32
    bf16 = mybir.dt.bfloat16
    Alu = mybir.AluOpType

    nc = bacc.Bacc("TRN2", target_bir_lowering=False, debug=False,
                   num_devices=NCORES)

    xT_in = nc.dram_tensor("xT", [IN_DIM, PADN], bf16, kind="ExternalInput")
    gidx_in = nc.dram_tensor("gidx", [128, G, S16], mybir.dt.int16,
                             kind="ExternalInput")
    S_in = nc.dram_tensor("S", [G, 128, C, 128], bf16, kind="ExternalInput")
    D_in = nc.dram_tensor("D", [G, 128, 128], bf16, kind="ExternalInput")
    Wc_in = nc.dram_tensor("Wc", [L, 2, 128, H], bf16, kind="ExternalInput")
    W_in_in = nc.dram_tensor("W_in", [IN_DIM, H], bf16, kind="ExternalInput")
    biases_in = nc.dram_tensor("biases", [L + 1, H], bf16,
                               kind="ExternalInput")
    ln_in = nc.dram_tensor("ln", [2 + 2 * L, H], f32, kind="ExternalInput")
    out_dram = nc.dram_tensor("out", [PADN, H], f32, kind="ExternalOutput")

    zbounces = [nc.dram_tensor(f"zbounce{l}", [PADN, H], bf16,
                               kind="Internal") for l in range(L)]
    # Shared output -> one-hop peer writes instead of RDH hops
    zfulls = [nc.dram_tensor(f"zfull{l}", [NCORES * PADN, H], bf16,
                             kind="Internal", addr_space="Shared")
              for l in range(L)]

    def bcast128(ap_row):
        return bass.AP(tensor=ap_row.tensor, offset=ap_row.offset,
                       ap=[[0, 128]] + list(ap_row.ap[1:]))

    with tile.TileContext(nc) as tc:
        with (
            tc.tile_pool(name="persist", bufs=1) as pp,
            tc.tile_pool(name="msgs_pool", bufs=4) as msgs_pool,
            tc.tile_pool(name="s_pool", bufs=4) as s_pool,
            tc.tile_pool(name="small", bufs=4) as small,
            tc.tile_pool(name="tiny", bufs=6) as tiny,
            tc.tile_pool(name="psum_a", bufs=2, space="PSUM") as psum_a,
            tc.tile_pool(name="psum_z", bufs=2, space="PSUM") as psum_z,
            tc.tile_pool(name="psum_t", bufs=2, space="PSUM") as psum_t,
        ):
            # ---------- persistent tiles ----------
            xcur = pp.tile([128, G, H], f32)
            h0s = pp.tile([128, G, H], f32)
            z_all = pp.tile([128, G, H], bf16)
            BLK = 4 if G % 4 == 0 else (2 if G % 2 == 0 else 1)
            NB = G // BLK
            msgs_all = pp.tile([128, 2, BLK * C, H], bf16)
            gidx_sb = pp.tile([128, G, S16], mybir.dt.int16)
            D_sb = pp.tile([128, G, 128], bf16)
            Wc_sb = pp.tile([128, L * 2, H], bf16)
            W_in_sb = pp.tile([128, H], bf16)
            bias_sb = pp.tile([1, L + 1, H], bf16)
            ones_sb = pp.tile([1, 128], bf16)
            ln_sb = pp.tile([128, 2 + 2 * L, H], f32)
            ident = pp.tile([128, 128], f32)
            xT_sb = pp.tile([128, PADN], bf16)

            nc.sync.dma_start(out=gidx_sb[:], in_=gidx_in.ap())
            for g in range(G):
                nc.sync.dma_start(out=D_sb[:, g, :], in_=D_in.ap()[g])
            for l in range(L):
                for kt in range(2):
                    nc.sync.dma_start(out=Wc_sb[:, l * 2 + kt, :],
                                      in_=Wc_in.ap()[l, kt])
            nc.sync.dma_start(out=W_in_sb[:], in_=W_in_in.ap())
            nc.sync.dma_start(out=bias_sb[:], in_=biases_in.ap()[None])
            nc.vector.memset(ones_sb[:], 1.0)
            for r in range(2 + 2 * L):
                nc.sync.dma_start(out=ln_sb[:, r, :],
                                  in_=bcast128(ln_in.ap()[r:r + 1, :]))
            make_identity(nc, ident[:])
            nc.sync.dma_start(out=xT_sb[:], in_=xT_in.ap())

            def quake_rstd(var_ap, bs):
                """rstd = 1/sqrt(var+eps) on DVE only (Newton, 2 iters)."""
                v = tiny.tile([128, BLK], f32, name="q_v")
                r = tiny.tile([128, BLK], f32, name="q_r")
                a = tiny.tile([128, BLK], f32, name="q_a")
                nc.vector.tensor_scalar_add(out=v[:, :bs], in0=var_ap,
                                            scalar1=float(EPS))
                vi = v[:, :bs].bitcast(i32)
                ri = r[:, :bs].bitcast(i32)
                nc.vector.tensor_scalar(out=ri, in0=vi, scalar1=1,
                                        scalar2=None,
                                        op0=Alu.logical_shift_right)
                nc.vector.tensor_scalar(out=ri, in0=ri, scalar1=-1,
                                        scalar2=QUAKE_MAGIC, op0=Alu.mult,
                                        op1=Alu.add)
                for _ in range(2):
                    nc.vector.tensor_tensor(out=a[:, :bs], in0=r[:, :bs],
                                            in1=r[:, :bs], op=Alu.mult)
                    nc.vector.tensor_tensor(out=a[:, :bs], in0=a[:, :bs],
                                            in1=v[:, :bs], op=Alu.mult)
                    nc.vector.tensor_scalar(out=a[:, :bs], in0=a[:, :bs],
                                            scalar1=-0.5, scalar2=1.5,
                                            op0=Alu.mult, op1=Alu.add)
                    nc.vector.tensor_tensor(out=r[:, :bs], in0=r[:, :bs],
                                            in1=a[:, :bs], op=Alu.mult)
                return r

            def gelu_stats(g, psum, y_blk, gsub, mv_blk):
                nc.scalar.activation(out=y_blk[:, gsub, :], in_=psum[:],
                                     func=ACT_FN)
                stats = tiny.tile([128, 6], f32, name="bn_st")
                nc.vector.bn_stats(out=stats[:], in_=y_blk[:, gsub, :])
                nc.vector.bn_aggr(out=mv_blk[:, gsub, :], in_=stats[:])

            def ln_blend_z(g, y_ap, mv_ap, rstd_ap, gi, bi, l, first):
                """LN + blend for group g, then z for layer l."""
                t = small.tile([128, H], f32, name="t_ln")
                nc.vector.tensor_scalar_sub(out=t[:], in0=y_ap,
                                            scalar1=mv_ap[0:128, 0:1])
                u = small.tile([128, H], f32, name="u_ln")
                nc.vector.scalar_tensor_tensor(
                    out=u[:], in0=t[:], scalar=rstd_ap, in1=ln_sb[:, gi, :],
                    op0=Alu.mult, op1=Alu.mult)
                if first:
                    nc.vector.tensor_tensor(out=xcur[:, g, :], in0=u[:],
                                            in1=ln_sb[:, bi, :], op=Alu.add)
                    nc.vector.tensor_scalar_mul(out=h0s[:, g, :],
                                                in0=xcur[:, g, :],
                                                scalar1=ALPHA)
                else:
                    v = small.tile([128, H], f32, name="v_ln")
                    nc.vector.tensor_tensor(out=v[:], in0=u[:],
                                            in1=ln_sb[:, bi, :], op=Alu.add)
                    w = small.tile([128, H], f32, name="w_ln")
                    nc.vector.tensor_tensor(out=w[:], in0=v[:],
                                            in1=h0s[:, g, :], op=Alu.add)
                    nc.vector.tensor_tensor(out=xcur[:, g, :],
                                            in0=xcur[:, g, :], in1=w[:],
                                            op=Alu.add)
                if l is not None:
                    # transpose xcur[g], z = xcur @ Wc[l] -> z_all + zbounce
                    tp = psum_t.tile([128, 2, 128], f32, name="tp")
                    xcurT = small.tile([128, 2, 128], bf16, name="xcurT")
                    for kt in range(2):
                        nc.tensor.transpose(
                            out=tp[:, kt, :],
                            in_=xcur[:, g, kt * 128:(kt + 1) * 128],
                            identity=ident[:])
                        nc.scalar.activation(
                            out=xcurT[:, kt, :], in_=tp[:, kt, :],
                            func=mybir.ActivationFunctionType.Copy)
                    zp = psum_z.tile([128, H], f32, name="zp")
                    for kt in range(2):
                        nc.tensor.matmul(
                            out=zp[:], lhsT=xcurT[:, kt, :],
                            rhs=Wc_sb[:, l * 2 + kt, :],
                            start=(kt == 0), stop=(kt == 1))
                    nc.scalar.activation(
                        out=z_all[:, g, :], in_=zp[:],
                        func=mybir.ActivationFunctionType.Copy)
                    nc.sync.dma_start(
                        out=zbounces[l].ap()[g * 128:(g + 1) * 128, :],
                        in_=z_all[:, g, :])
                else:
                    nc.sync.dma_start(
                        out=out_dram.ap()[g * 128:(g + 1) * 128, :],
                        in_=xcur[:, g, :])

            # clear msgs buffers once: -1 pad slots are never written by the
            # gather, and S weights of 0 must multiply finite values
            nc.vector.memset(msgs_all[:], 0.0)

            # ---------- input block ----------
            for b in range(NB):
                y_blk = small.tile([128, BLK, H], f32, name="y_blk")
                mv_blk = tiny.tile([128, BLK, 2], f32, name="mv_blk")
                for gsub in range(BLK):
                    g = b * BLK + gsub
                    hp = psum_a.tile([128, H], f32, name="agg")
                    nc.tensor.matmul(out=hp[:],
                                     lhsT=xT_sb[:, g * 128:(g + 1) * 128],
                                     rhs=W_in_sb[:], start=True, stop=False)
                    nc.tensor.matmul(out=hp[:], lhsT=ones_sb[:],
                                     rhs=bias_sb[:, 0, :], start=False,
                                     stop=True)
                    gelu_stats(g, hp, y_blk, gsub, mv_blk)
                rstd = quake_rstd(mv_blk[:, :, 1], BLK)
                for gsub in range(BLK):
                    g = b * BLK + gsub
                    ln_blend_z(g, y_blk[:, gsub, :], mv_blk[:, gsub, :],
                               rstd[:, gsub:gsub + 1], 0, 1, 0, first=True)

            for l in range(L):
                nc.gpsimd.collective_compute(
                    "AllGather", mybir.AluOpType.bypass,
                    replica_groups=[list(range(NCORES))],
                    ins=[zbounces[l].ap()], outs=[zfulls[l].ap()])

                for b in range(NB):
                    buf = (l * NB + b) % 2
                    msgs = msgs_all[:, buf, :, :]
                    nc.gpsimd.dma_gather(
                        msgs, zfulls[l].ap(),
                        gidx_sb[:, b * BLK:(b + 1) * BLK, :],
                        num_idxs=BLK * SLOTS, num_idxs_reg=BLK * SLOTS,
                        elem_size=H, single_packet=False)
                    y_blk = small.tile([128, BLK, H], f32, name="y_blk")
                    mv_blk = tiny.tile([128, BLK, 2], f32, name="mv_blk")
                    for gsub in range(BLK):
                        g = b * BLK + gsub
                        s_sb = s_pool.tile([128, C, 128], bf16, name="s_sb")
                        nc.sync.dma_start(out=s_sb[:], in_=S_in.ap()[g])
                        agg = psum_a.tile([128, H], f32, name="agg")
                        # self-loop diagonal first
                        nc.tensor.matmul(out=agg[:], lhsT=D_sb[:, g, :],
                                         rhs=z_all[:, g, :], start=True,
                                         stop=False)
                        for c in range(C):
                            nc.tensor.matmul(
                                out=agg[:], lhsT=s_sb[:, c, :],
                                rhs=msgs[:, gsub * C + c, :],
                                start=False, stop=False)
                        nc.tensor.matmul(
                            out=agg[:], lhsT=ones_sb[:],
                            rhs=bias_sb[:, 1 + l, :],
                            start=False, stop=True)
                        gelu_stats(g, agg, y_blk, gsub, mv_blk)
                    rstd = quake_rstd(mv_blk[:, :, 1], BLK)
                    for gsub in range(BLK):
                        g = b * BLK + gsub
                        ln_blend_z(g, y_blk[:, gsub, :], mv_blk[:, gsub, :],
                                   rstd[:, gsub:gsub + 1], 2 + l, 2 + L + l,
                                   l + 1 if l < L - 1 else None, first=False)

    nc.compile()
    return nc


def _get_program(cfg, C):
    key = (cfg, C)
    if key not in _cache:
        _cache[key] = _build(cfg, C)
    return _cache[key]


def run_sharded(inputs, trace=False, cfg=DEFAULT_CFG):
    in_maps, C = _preprocess(cfg, **inputs)
    nc = _get_program(cfg, C)
    res = bass_utils.run_bass_kernel_spmd(
        nc, in_maps, core_ids=list(range(NCORES)), trace=trace)
    out = np.empty((cfg.n, H), dtype=np.float32)
    for c in range(NCORES):
        out[c * cfg.percore:(c + 1) * cfg.percore] = \
            res.results[c]["out"][:cfg.percore]
    return out, res


def kernel(**inputs):
    out, _ = run_sharded(inputs, trace=False)
    return out


# revision 31
# speedup vs baseline: 1.2154x; 1.2154x over previous
"""GCN encoder kernel for Trainium2 (8 NeuronCores).

Strategy (graph/data parallel, per sharding hint):
  - Nodes sharded by destination range across 8 cores (2500 -> padded 2560/core).
  - Host precomputes GCN symmetric normalization and a per-destination-group
    schedule: non-self edges are deduplicated by source per 128-dst group and
    padded with trailing -1 (skipped by the gather ucode).  The segment-sum
    becomes dense matmuls with tiny scatter matrices S[src_slot, dst] holding
    the summed edge norms; self-loops are applied as a diagonal matmul against
    the locally-kept z.
  - Per layer: AllGather z (bf16) across cores, dma_gather the unique source
    rows, TensorE matmuls accumulate messages into PSUM per dst group (with the
    per-layer bias folded in as a rank-1 matmul), then GELU + LayerNorm
    (rsqrt via Newton on VectorE; no ACT table switches) + residual blend.
"""

import sys
from dataclasses import dataclass

sys.path.insert(0, "/opt/trn_rl_repo")

import numpy as np
import ml_dtypes

import concourse.bass as bass
import concourse.tile as tile
from concourse import bacc, mybir
from concourse import bass_utils
from concourse.masks import make_identity

BF16 = ml_dtypes.bfloat16
ALPHA = 0.1
EPS = 1e-5
IN_DIM = 128
H = 256
NCORES = 8
ACT_FN = mybir.ActivationFunctionType.Gelu_apprx_tanh
QUAKE_MAGIC = 0x5F3759DF


@dataclass(frozen=True)
class Cfg:
    n: int = 20000
    layers: int = 6

    @property
    def percore(self):
        return self.n // NCORES

    @property
    def groups(self):
        return (self.percore + 127) // 128

    @property
    def padn(self):
        return self.groups * 128


DEFAULT_CFG = Cfg()
_cache = {}


def _preprocess(cfg, x, edge_index, W_in, b_in, g_in, beta_in, Wc, bc, gc,
                betac):
    """Host-side graph preprocessing -> per-core input maps."""
    N, G, PERCORE, PADN, L = (cfg.n, cfg.groups, cfg.percore, cfg.padn,
                              cfg.layers)
    x = np.asarray(x, dtype=np.float32)
    ei = np.asarray(edge_index).astype(np.int64)
    W_in = np.asarray(W_in, dtype=np.float32)
    b_in = np.asarray(b_in, dtype=np.float32)
    g_in = np.asarray(g_in, dtype=np.float32)
    beta_in = np.asarray(beta_in, dtype=np.float32)
    Wc = np.asarray(Wc, dtype=np.float32)
    bc = np.asarray(bc, dtype=np.float32)
    gc = np.asarray(gc, dtype=np.float32)
    betac = np.asarray(betac, dtype=np.float32)

    loop = np.arange(N, dtype=np.int64)
    col_all = np.concatenate([ei[1], loop])   # dst (for degree)
    deg = np.bincount(col_all, minlength=N).astype(np.float32)
    dinv = np.where(deg > 0, 1.0 / np.sqrt(deg), 0.0).astype(np.float32)

    # non-self edges (self loops handled by the diagonal matmul)
    row = ei[0]
    col = ei[1]
    norm = (dinv[row] * dinv[col]).astype(np.float32)
    pad_src = (row // PERCORE) * PADN + (row % PERCORE)

    core_of = col // PERCORE
    dloc_all = col % PERCORE
    grp_all = dloc_all // 128
    dst_all = dloc_all % 128

    per_core = []
    maxU = 1
    for c in range(NCORES):
        m = core_of == c
        g_e = grp_all[m]
        d_e = dst_all[m]
        s_e = pad_src[m]
        n_e = norm[m]
        # dedup sources within each group
        key = g_e * (NCORES * PADN) + s_e
        ukey, slot_of_edge = np.unique(key, return_inverse=True)
        u_grp = ukey // (NCORES * PADN)
        u_src = ukey % (NCORES * PADN)
        counts = np.bincount(u_grp, minlength=G)
        maxU = max(maxU, counts.max())
        per_core.append((g_e, d_e, s_e, n_e, slot_of_edge, u_grp, u_src,
                         counts))

    C = int((maxU + 127) // 128)
    SLOTS = C * 128
    S16 = SLOTS // 16

    in_maps = []
    for c in range(NCORES):
        g_e, d_e, s_e, n_e, slot_of_edge, u_grp, u_src, counts = per_core[c]
        starts = np.zeros(G + 1, dtype=np.int64)
        np.cumsum(counts, out=starts[1:])
        # slot of each unique (group, src) within its group
        u_slot = np.arange(len(u_grp)) - starts[u_grp]

        # Pads are dummy index 0 (gathered but weighted 0 in S): groups are
        # batched 4-per-gather-instruction, so pads sit interior where the
        # ucode's trailing -1 trim cannot apply.
        idx_l = np.zeros((G, SLOTS), dtype=np.int16)
        idx_l[u_grp, u_slot] = u_src.astype(np.int16)

        S_l = np.zeros((G, SLOTS, 128), dtype=np.float32)
        np.add.at(S_l, (g_e, u_slot[slot_of_edge], d_e), n_e)

        # idx layout: logical slot i -> partition 16*stripe + i%16, col i//16
        idx_rs = idx_l.reshape(G, S16, 16).transpose(2, 0, 1)   # [16, G, S16]
        gidx = np.tile(idx_rs, (8, 1, 1)).astype(np.int16)      # [128, G, S16]

        # S layout: [G, 128(slot%128), C(chunk), 128(dst)]
        S_arr = S_l.reshape(G, C, 128, 128).transpose(0, 2, 1, 3).astype(BF16)

        # self-loop diagonal: D[g, p, p] = dinv[node]^2
        nodes = np.arange(PERCORE) + c * PERCORE
        d2 = np.zeros(PADN, dtype=np.float32)
        d2[:PERCORE] = dinv[nodes] ** 2
        D_arr = np.zeros((G, 128, 128), dtype=np.float32)
        pi = np.arange(128)
        for g in range(G):
            D_arr[g, pi, pi] = d2[g * 128:(g + 1) * 128]
        D_arr = D_arr.astype(BF16)

        xs = x[c * PERCORE:(c + 1) * PERCORE]
        xT = np.zeros((IN_DIM, PADN), dtype=BF16)
        xT[:, :PERCORE] = xs.T.astype(BF16)

        in_maps.append({
            "xT": np.ascontiguousarray(xT),
            "gidx": np.ascontiguousarray(gidx),
            "S": np.ascontiguousarray(S_arr),
            "D": np.ascontiguousarray(D_arr),
        })

    Wc_bf = np.ascontiguousarray(Wc.reshape(L, 2, 128, H).astype(BF16))
    W_in_bf = W_in.astype(BF16)
    biases = np.concatenate([b_in[None, :], bc], axis=0).astype(BF16)
    ln = np.zeros((2 + 2 * L, H), dtype=np.float32)
    ln[0] = g_in
    ln[1] = beta_in
    ln[2:2 + L] = (1.0 - ALPHA) * gc
    ln[2 + L:2 + 2 * L] = (1.0 - ALPHA) * betac
    for m in in_maps:
        m["Wc"] = Wc_bf
        m["W_in"] = W_in_bf
        m["biases"] = biases
        m["ln"] = ln

    return in_maps, C


def _build(cfg, C):
    """Build the Bass program (shared by all 8 cores)."""
    G, PADN, L = cfg.groups, cfg.padn, cfg.layers
    SLOTS = C * 128
    S16 = SLOTS // 16
    f32 = mybir.dt.float32
    i32 = mybir.dt.int32
    bf16 = mybir.dt.bfloat16
    Alu = mybir.AluOpType

    nc = bacc.Bacc("TRN2", target_bir_lowering=False, debug=False,
                   num_devices=NCORES)

    xT_in = nc.dram_tensor("xT", [IN_DIM, PADN], bf16, kind="ExternalInput")
    gidx_in = nc.dram_tensor("gidx", [128, G, S16], mybir.dt.int16,
                             kind="ExternalInput")
    S_in = nc.dram_tensor("S", [G, 128, C, 128], bf16, kind="ExternalInput")
    D_in = nc.dram_tensor("D", [G, 128, 128], bf16, kind="ExternalInput")
    Wc_in = nc.dram_tensor("Wc", [L, 2, 128, H], bf16, kind="ExternalInput")
    W_in_in = nc.dram_tensor("W_in", [IN_DIM, H], bf16, kind="ExternalInput")
    biases_in = nc.dram_tensor("biases", [L + 1, H], bf16,
                               kind="ExternalInput")
    ln_in = nc.dram_tensor("ln", [2 + 2 * L, H], f32, kind="ExternalInput")
    out_dram = nc.dram_tensor("out", [PADN, H], f32, kind="ExternalOutput")

    zbounces = [nc.dram_tensor(f"zbounce{l}", [PADN, H], bf16,
                               kind="Internal") for l in range(L)]
    # Shared output -> one-hop peer writes instead of RDH hops
    zfulls = [nc.dram_tensor(f"zfull{l}", [NCORES * PADN, H], bf16,
                             kind="Internal", addr_space="Shared")
              for l in range(L)]

    def bcast128(ap_row):
        return bass.AP(tensor=ap_row.tensor, offset=ap_row.offset,
                       ap=[[0, 128]] + list(ap_row.ap[1:]))

    with tile.TileContext(nc) as tc:
        with (
            tc.tile_pool(name="persist", bufs=1) as pp,
            tc.tile_pool(name="msgs_pool", bufs=4) as msgs_pool,
            tc.tile_pool(name="s_pool", bufs=4) as s_pool,
            tc.tile_pool(name="small", bufs=4) as small,
            tc.tile_pool(name="tiny", bufs=6) as tiny,
            tc.tile_pool(name="psum_a", bufs=2, space="PSUM") as psum_a,
            tc.tile_pool(name="psum_z", bufs=2, space="PSUM") as psum_z,
            tc.tile_pool(name="psum_t", bufs=2, space="PSUM") as psum_t,
        ):
            # ---------- persistent tiles ----------
            xcur = pp.tile([128, G, H], f32)
            h0s = pp.tile([128, G, H], f32)
            z_all = pp.tile([128, G, H], bf16)
            BLK = 2 if G % 2 == 0 else 1
            NB = G // BLK
            MB = 3
            msgs_all = pp.tile([128, MB, BLK * C, H], bf16)
            y_all = pp.tile([128, G, H], bf16)
            mv_all = pp.tile([128, G, 2], f32)
            rstd_all = pp.tile([128, G], f32)
            eps_sb = pp.tile([128, 1], f32)
            nc.vector.memset(eps_sb[:], EPS)
            gidx_sb = pp.tile([128, G, S16], mybir.dt.int16)
            D_sb = pp.tile([128, G, 128], bf16)
            Wc_sb = pp.tile([128, L * 2, H], bf16)
            W_in_sb = pp.tile([128, H], bf16)
            bias_sb = pp.tile([1, L + 1, H], bf16)
            ones_sb = pp.tile([1, 128], bf16)
            ln_sb = pp.tile([128, 2 + 2 * L, H], f32)
            ident = pp.tile([128, 128], f32)
            xT_sb = pp.tile([128, PADN], bf16)

            nc.sync.dma_start(out=gidx_sb[:], in_=gidx_in.ap())
            for g in range(G):
                nc.sync.dma_start(out=D_sb[:, g, :], in_=D_in.ap()[g])
            for l in range(L):
                for kt in range(2):
                    nc.sync.dma_start(out=Wc_sb[:, l * 2 + kt, :],
                                      in_=Wc_in.ap()[l, kt])
            nc.sync.dma_start(out=W_in_sb[:], in_=W_in_in.ap())
            nc.sync.dma_start(out=bias_sb[:], in_=biases_in.ap()[None])
            nc.vector.memset(ones_sb[:], 1.0)
            for r in range(2 + 2 * L):
                nc.sync.dma_start(out=ln_sb[:, r, :],
                                  in_=bcast128(ln_in.ap()[r:r + 1, :]))
            make_identity(nc, ident[:])
            nc.sync.dma_start(out=xT_sb[:], in_=xT_in.ap())

            def batched_rstd():
                nc.scalar.activation(
                    out=rstd_all[:], in_=mv_all[:, :, 1],
                    func=mybir.ActivationFunctionType.Sqrt, bias=eps_sb[:])
                nc.vector.reciprocal(out=rstd_all[:], in_=rstd_all[:])

            def gelu_stats(g, psum):
                nc.scalar.activation(out=y_all[:, g, :], in_=psum[:],
                                     func=ACT_FN)
                stats = tiny.tile([128, 6], f32, name="bn_st")
                nc.vector.bn_stats(out=stats[:], in_=y_all[:, g, :])
                nc.vector.bn_aggr(out=mv_all[:, g, :], in_=stats[:])

            def ln_blend_z(g, y_ap, mv_ap, rstd_ap, gi, bi, l, first):
                """LN + blend for group g, then z for layer l."""
                t = small.tile([128, H], f32, name="t_ln")
                nc.vector.tensor_scalar_sub(out=t[:], in0=y_ap,
                                            scalar1=mv_ap[0:128, 0:1])
                u = small.tile([128, H], f32, name="u_ln")
                nc.vector.scalar_tensor_tensor(
                    out=u[:], in0=t[:], scalar=rstd_ap, in1=ln_sb[:, gi, :],
                    op0=Alu.mult, op1=Alu.mult)
                if first:
                    nc.vector.tensor_tensor(out=xcur[:, g, :], in0=u[:],
                                            in1=ln_sb[:, bi, :], op=Alu.add)
                    nc.vector.tensor_scalar_mul(out=h0s[:, g, :],
                                                in0=xcur[:, g, :],
                                                scalar1=ALPHA)
                else:
                    v = small.tile([128, H], f32, name="v_ln")
                    nc.vector.tensor_tensor(out=v[:], in0=u[:],
                                            in1=ln_sb[:, bi, :], op=Alu.add)
                    w = small.tile([128, H], f32, name="w_ln")
                    nc.vector.tensor_tensor(out=w[:], in0=v[:],
                                            in1=h0s[:, g, :], op=Alu.add)
                    nc.vector.tensor_tensor(out=xcur[:, g, :],
                                            in0=xcur[:, g, :], in1=w[:],
                                            op=Alu.add)
                if l is not None:
                    # transpose xcur[g], z = xcur @ Wc[l] -> z_all + zbounce
                    tp = psum_t.tile([128, 2, 128], f32, name="tp")
                    xcurT = small.tile([128, 2, 128], bf16, name="xcurT")
                    for kt in range(2):
                        nc.tensor.transpose(
                            out=tp[:, kt, :],
                            in_=xcur[:, g, kt * 128:(kt + 1) * 128],
                            identity=ident[:])
                        nc.scalar.activation(
                            out=xcurT[:, kt, :], in_=tp[:, kt, :],
                            func=mybir.ActivationFunctionType.Copy)
                    zp = psum_z.tile([128, H], f32, name="zp")
                    for kt in range(2):
                        nc.tensor.matmul(
                            out=zp[:], lhsT=xcurT[:, kt, :],
                            rhs=Wc_sb[:, l * 2 + kt, :],
                            start=(kt == 0), stop=(kt == 1))
                    nc.scalar.activation(
                        out=z_all[:, g, :], in_=zp[:],
                        func=mybir.ActivationFunctionType.Copy)
                    nc.sync.dma_start(
                        out=zbounces[l].ap()[g * 128:(g + 1) * 128, :],
                        in_=z_all[:, g, :])
                else:
                    nc.sync.dma_start(
                        out=out_dram.ap()[g * 128:(g + 1) * 128, :],
                        in_=xcur[:, g, :])

            # clear msgs buffers once: -1 pad slots are never written by the
            # gather, and S weights of 0 must multiply finite values
            nc.vector.memset(msgs_all[:], 0.0)

            # ---------- input block ----------
            for g in range(G):
                hp = psum_a.tile([128, H], f32, name="agg")
                nc.tensor.matmul(out=hp[:],
                                 lhsT=xT_sb[:, g * 128:(g + 1) * 128],
                                 rhs=W_in_sb[:], start=True, stop=False)
                nc.tensor.matmul(out=hp[:], lhsT=ones_sb[:],
                                 rhs=bias_sb[:, 0, :], start=False,
                                 stop=True)
                gelu_stats(g, hp)
            batched_rstd()
            for g in range(G):
                ln_blend_z(g, y_all[:, g, :], mv_all[:, g, :],
                           rstd_all[:, g:g + 1], 0, 1, 0, first=True)

            for l in range(L):
                nc.gpsimd.collective_compute(
                    "AllGather", mybir.AluOpType.bypass,
                    replica_groups=[list(range(NCORES))],
                    ins=[zbounces[l].ap()], outs=[zfulls[l].ap()])

                for b in range(NB):
                    buf = (l * NB + b) % MB
                    msgs = msgs_all[:, buf, :, :]
                    nc.gpsimd.dma_gather(
                        msgs, zfulls[l].ap(),
                        gidx_sb[:, b * BLK:(b + 1) * BLK, :],
                        num_idxs=BLK * SLOTS, num_idxs_reg=BLK * SLOTS,
                        elem_size=H, single_packet=False)
                    for gsub in range(BLK):
                        g = b * BLK + gsub
                        s_sb = s_pool.tile([128, C, 128], bf16, name="s_sb")
                        nc.sync.dma_start(out=s_sb[:], in_=S_in.ap()[g])
                        agg = psum_a.tile([128, H], f32, name="agg")
                        # self-loop diagonal first
                        nc.tensor.matmul(out=agg[:], lhsT=D_sb[:, g, :],
                                         rhs=z_all[:, g, :], start=True,
                                         stop=False)
                        for c in range(C):
                            nc.tensor.matmul(
                                out=agg[:], lhsT=s_sb[:, c, :],
                                rhs=msgs[:, gsub * C + c, :],
                                start=False, stop=False)
                        nc.tensor.matmul(
                            out=agg[:], lhsT=ones_sb[:],
                            rhs=bias_sb[:, 1 + l, :],
                            start=False, stop=True)
                        gelu_stats(g, agg)
                batched_rstd()
                for g in range(G):
                    ln_blend_z(g, y_all[:, g, :], mv_all[:, g, :],
                               rstd_all[:, g:g + 1], 2 + l, 2 + L + l,
                               l + 1 if l < L - 1 else None, first=False)

    nc.compile()
    return nc


def _get_program(cfg, C):
    key = (cfg, C)
    if key not in _cache:
        _cache[key] = _build(cfg, C)
    return _cache[key]


def run_sharded(inputs, trace=False, cfg=DEFAULT_CFG):
    in_maps, C = _preprocess(cfg, **inputs)
    nc = _get_program(cfg, C)
    res = bass_utils.run_bass_kernel_spmd(
        nc, in_maps, core_ids=list(range(NCORES)), trace=trace)
    out = np.empty((cfg.n, H), dtype=np.float32)
    for c in range(NCORES):
        out[c * cfg.percore:(c + 1) * cfg.percore] = \
            res.results[c]["out"][:cfg.percore]
    return out, res


def kernel(**inputs):
    out, _ = run_sharded(inputs, trace=False)
    return out


# revision 32
# speedup vs baseline: 1.2234x; 1.0066x over previous
"""GCN encoder kernel for Trainium2 (8 NeuronCores).

Strategy (graph/data parallel, per sharding hint):
  - Nodes sharded by destination range across 8 cores (2500 -> padded 2560/core).
  - Host precomputes GCN symmetric normalization and a per-destination-group
    schedule: non-self edges are deduplicated by source per 128-dst group and
    padded with trailing -1 (skipped by the gather ucode).  The segment-sum
    becomes dense matmuls with tiny scatter matrices S[src_slot, dst] holding
    the summed edge norms; self-loops are applied as a diagonal matmul against
    the locally-kept z.
  - Per layer: AllGather z (bf16) across cores, dma_gather the unique source
    rows, TensorE matmuls accumulate messages into PSUM per dst group (with the
    per-layer bias folded in as a rank-1 matmul), then GELU + LayerNorm
    (rsqrt via Newton on VectorE; no ACT table switches) + residual blend.
"""

import sys
from dataclasses import dataclass

sys.path.insert(0, "/opt/trn_rl_repo")

import numpy as np
import ml_dtypes

import concourse.bass as bass
import concourse.tile as tile
from concourse import bacc, mybir
from concourse import bass_utils
from concourse.masks import make_identity

BF16 = ml_dtypes.bfloat16
ALPHA = 0.1
EPS = 1e-5
IN_DIM = 128
H = 256
NCORES = 8
ACT_FN = mybir.ActivationFunctionType.Gelu_apprx_tanh
QUAKE_MAGIC = 0x5F3759DF


@dataclass(frozen=True)
class Cfg:
    n: int = 20000
    layers: int = 6

    @property
    def percore(self):
        return self.n // NCORES

    @property
    def groups(self):
        return (self.percore + 127) // 128

    @property
    def padn(self):
        return self.groups * 128


DEFAULT_CFG = Cfg()
_cache = {}


def _preprocess(cfg, x, edge_index, W_in, b_in, g_in, beta_in, Wc, bc, gc,
                betac):
    """Host-side graph preprocessing -> per-core input maps."""
    N, G, PERCORE, PADN, L = (cfg.n, cfg.groups, cfg.percore, cfg.padn,
                              cfg.layers)
    x = np.asarray(x, dtype=np.float32)
    ei = np.asarray(edge_index).astype(np.int64)
    W_in = np.asarray(W_in, dtype=np.float32)
    b_in = np.asarray(b_in, dtype=np.float32)
    g_in = np.asarray(g_in, dtype=np.float32)
    beta_in = np.asarray(beta_in, dtype=np.float32)
    Wc = np.asarray(Wc, dtype=np.float32)
    bc = np.asarray(bc, dtype=np.float32)
    gc = np.asarray(gc, dtype=np.float32)
    betac = np.asarray(betac, dtype=np.float32)

    loop = np.arange(N, dtype=np.int64)
    col_all = np.concatenate([ei[1], loop])   # dst (for degree)
    deg = np.bincount(col_all, minlength=N).astype(np.float32)
    dinv = np.where(deg > 0, 1.0 / np.sqrt(deg), 0.0).astype(np.float32)

    # non-self edges (self loops handled by the diagonal matmul)
    row = ei[0]
    col = ei[1]
    norm = (dinv[row] * dinv[col]).astype(np.float32)
    pad_src = (row // PERCORE) * PADN + (row % PERCORE)

    core_of = col // PERCORE
    dloc_all = col % PERCORE
    grp_all = dloc_all // 128
    dst_all = dloc_all % 128

    per_core = []
    maxU = 1
    for c in range(NCORES):
        m = core_of == c
        g_e = grp_all[m]
        d_e = dst_all[m]
        s_e = pad_src[m]
        n_e = norm[m]
        # dedup sources within each group
        key = g_e * (NCORES * PADN) + s_e
        ukey, slot_of_edge = np.unique(key, return_inverse=True)
        u_grp = ukey // (NCORES * PADN)
        u_src = ukey % (NCORES * PADN)
        counts = np.bincount(u_grp, minlength=G)
        maxU = max(maxU, counts.max())
        per_core.append((g_e, d_e, s_e, n_e, slot_of_edge, u_grp, u_src,
                         counts))

    C = int((maxU + 127) // 128)
    SLOTS = C * 128
    S16 = SLOTS // 16

    in_maps = []
    for c in range(NCORES):
        g_e, d_e, s_e, n_e, slot_of_edge, u_grp, u_src, counts = per_core[c]
        starts = np.zeros(G + 1, dtype=np.int64)
        np.cumsum(counts, out=starts[1:])
        # slot of each unique (group, src) within its group
        u_slot = np.arange(len(u_grp)) - starts[u_grp]

        # Pads are dummy index 0 (gathered but weighted 0 in S): groups are
        # batched 4-per-gather-instruction, so pads sit interior where the
        # ucode's trailing -1 trim cannot apply.
        idx_l = np.zeros((G, SLOTS), dtype=np.int16)
        idx_l[u_grp, u_slot] = u_src.astype(np.int16)

        S_l = np.zeros((G, SLOTS, 128), dtype=np.float32)
        np.add.at(S_l, (g_e, u_slot[slot_of_edge], d_e), n_e)

        # idx layout: logical slot i -> partition 16*stripe + i%16, col i//16
        idx_rs = idx_l.reshape(G, S16, 16).transpose(2, 0, 1)   # [16, G, S16]
        gidx = np.tile(idx_rs, (8, 1, 1)).astype(np.int16)      # [128, G, S16]

        # S layout: [G, 128(slot%128), C(chunk), 128(dst)]
        S_arr = S_l.reshape(G, C, 128, 128).transpose(0, 2, 1, 3).astype(BF16)

        # self-loop diagonal: D[g, p, p] = dinv[node]^2
        nodes = np.arange(PERCORE) + c * PERCORE
        d2 = np.zeros(PADN, dtype=np.float32)
        d2[:PERCORE] = dinv[nodes] ** 2
        D_arr = np.zeros((G, 128, 128), dtype=np.float32)
        pi = np.arange(128)
        for g in range(G):
            D_arr[g, pi, pi] = d2[g * 128:(g + 1) * 128]
        D_arr = D_arr.astype(BF16)

        xs = x[c * PERCORE:(c + 1) * PERCORE]
        xT = np.zeros((IN_DIM, PADN), dtype=BF16)
        xT[:, :PERCORE] = xs.T.astype(BF16)

        in_maps.append({
            "xT": np.ascontiguousarray(xT),
            "gidx": np.ascontiguousarray(gidx),
            "S": np.ascontiguousarray(S_arr),
            "D": np.ascontiguousarray(D_arr),
        })

    Wc_bf = np.ascontiguousarray(Wc.reshape(L, 2, 128, H).astype(BF16))
    W_in_bf = W_in.astype(BF16)
    biases = np.concatenate([b_in[None, :], bc], axis=0).astype(BF16)
    ln = np.zeros((2 + 2 * L, H), dtype=np.float32)
    ln[0] = g_in
    ln[1] = beta_in
    ln[2:2 + L] = (1.0 - ALPHA) * gc
    ln[2 + L:2 + 2 * L] = (1.0 - ALPHA) * betac
    for m in in_maps:
        m["Wc"] = Wc_bf
        m["W_in"] = W_in_bf
        m["biases"] = biases
        m["ln"] = ln

    return in_maps, C


def _build(cfg, C):
    """Build the Bass program (shared by all 8 cores)."""
    G, PADN, L = cfg.groups, cfg.padn, cfg.layers
    SLOTS = C * 128
    S16 = SLOTS // 16
    f32 = mybir.dt.float32
    i32 = mybir.dt.int32
    bf16 = mybir.dt.bfloat16
    Alu = mybir.AluOpType

    nc = bacc.Bacc("TRN2", target_bir_lowering=False, debug=False,
                   num_devices=NCORES)

    xT_in = nc.dram_tensor("xT", [IN_DIM, PADN], bf16, kind="ExternalInput")
    gidx_in = nc.dram_tensor("gidx", [128, G, S16], mybir.dt.int16,
                             kind="ExternalInput")
    S_in = nc.dram_tensor("S", [G, 128, C, 128], bf16, kind="ExternalInput")
    D_in = nc.dram_tensor("D", [G, 128, 128], bf16, kind="ExternalInput")
    Wc_in = nc.dram_tensor("Wc", [L, 2, 128, H], bf16, kind="ExternalInput")
    W_in_in = nc.dram_tensor("W_in", [IN_DIM, H], bf16, kind="ExternalInput")
    biases_in = nc.dram_tensor("biases", [L + 1, H], bf16,
                               kind="ExternalInput")
    ln_in = nc.dram_tensor("ln", [2 + 2 * L, H], f32, kind="ExternalInput")
    out_dram = nc.dram_tensor("out", [PADN, H], f32, kind="ExternalOutput")

    zbounces = [nc.dram_tensor(f"zbounce{l}", [PADN, H], bf16,
                               kind="Internal") for l in range(L)]
    # Shared output -> one-hop peer writes instead of RDH hops
    zfulls = [nc.dram_tensor(f"zfull{l}", [NCORES * PADN, H], bf16,
                             kind="Internal", addr_space="Shared")
              for l in range(L)]

    def bcast128(ap_row):
        return bass.AP(tensor=ap_row.tensor, offset=ap_row.offset,
                       ap=[[0, 128]] + list(ap_row.ap[1:]))

    with tile.TileContext(nc) as tc:
        with (
            tc.tile_pool(name="persist", bufs=1) as pp,
            tc.tile_pool(name="msgs_pool", bufs=4) as msgs_pool,
            tc.tile_pool(name="s_pool", bufs=6) as s_pool,
            tc.tile_pool(name="small", bufs=4) as small,
            tc.tile_pool(name="tiny", bufs=6) as tiny,
            tc.tile_pool(name="psum_a", bufs=3, space="PSUM") as psum_a,
            tc.tile_pool(name="psum_z", bufs=2, space="PSUM") as psum_z,
            tc.tile_pool(name="psum_t", bufs=2, space="PSUM") as psum_t,
        ):
            # ---------- persistent tiles ----------
            xcur = pp.tile([128, G, H], f32)
            h0s = pp.tile([128, G, H], f32)
            z_all = pp.tile([128, G, H], bf16)
            BLK = 2 if G % 2 == 0 else 1
            NB = G // BLK
            MB = 4
            msgs_all = pp.tile([128, MB, BLK * C, H], bf16)
            y_all = pp.tile([128, G, H], bf16)
            mv_all = pp.tile([128, G, 2], f32)
            rstd_all = pp.tile([128, G], f32)
            eps_sb = pp.tile([128, 1], f32)
            nc.vector.memset(eps_sb[:], EPS)
            gidx_sb = pp.tile([128, G, S16], mybir.dt.int16)
            D_sb = pp.tile([128, G, 128], bf16)
            Wc_sb = pp.tile([128, L * 2, H], bf16)
            W_in_sb = pp.tile([128, H], bf16)
            bias_sb = pp.tile([1, L + 1, H], bf16)
            ones_sb = pp.tile([1, 128], bf16)
            ln_sb = pp.tile([128, 2 + 2 * L, H], f32)
            ident = pp.tile([128, 128], f32)
            xT_sb = pp.tile([128, PADN], bf16)

            nc.sync.dma_start(out=gidx_sb[:], in_=gidx_in.ap())
            for g in range(G):
                nc.sync.dma_start(out=D_sb[:, g, :], in_=D_in.ap()[g])
            for l in range(L):
                for kt in range(2):
                    nc.sync.dma_start(out=Wc_sb[:, l * 2 + kt, :],
                                      in_=Wc_in.ap()[l, kt])
            nc.sync.dma_start(out=W_in_sb[:], in_=W_in_in.ap())
            nc.sync.dma_start(out=bias_sb[:], in_=biases_in.ap()[None])
            nc.vector.memset(ones_sb[:], 1.0)
            for r in range(2 + 2 * L):
                nc.sync.dma_start(out=ln_sb[:, r, :],
                                  in_=bcast128(ln_in.ap()[r:r + 1, :]))
            make_identity(nc, ident[:])
            nc.sync.dma_start(out=xT_sb[:], in_=xT_in.ap())

            def batched_rstd():
                nc.scalar.activation(
                    out=rstd_all[:], in_=mv_all[:, :, 1],
                    func=mybir.ActivationFunctionType.Sqrt, bias=eps_sb[:])
                nc.vector.reciprocal(out=rstd_all[:], in_=rstd_all[:])

            def gelu_stats(g, psum):
                nc.scalar.activation(out=y_all[:, g, :], in_=psum[:],
                                     func=ACT_FN)
                stats = tiny.tile([128, 6], f32, name="bn_st")
                nc.vector.bn_stats(out=stats[:], in_=y_all[:, g, :])
                nc.vector.bn_aggr(out=mv_all[:, g, :], in_=stats[:])

            def ln_blend_z(g, y_ap, mv_ap, rstd_ap, gi, bi, l, first):
                """LN + blend for group g, then z for layer l."""
                t = small.tile([128, H], f32, name="t_ln")
                nc.vector.tensor_scalar_sub(out=t[:], in0=y_ap,
                                            scalar1=mv_ap[0:128, 0:1])
                u = small.tile([128, H], f32, name="u_ln")
                nc.vector.scalar_tensor_tensor(
                    out=u[:], in0=t[:], scalar=rstd_ap, in1=ln_sb[:, gi, :],
                    op0=Alu.mult, op1=Alu.mult)
                if first:
                    nc.vector.tensor_tensor(out=xcur[:, g, :], in0=u[:],
                                            in1=ln_sb[:, bi, :], op=Alu.add)
                    nc.vector.tensor_scalar_mul(out=h0s[:, g, :],
                                                in0=xcur[:, g, :],
                                                scalar1=ALPHA)
                else:
                    v = small.tile([128, H], f32, name="v_ln")
                    nc.vector.tensor_tensor(out=v[:], in0=u[:],
                                            in1=ln_sb[:, bi, :], op=Alu.add)
                    w = small.tile([128, H], f32, name="w_ln")
                    nc.vector.tensor_tensor(out=w[:], in0=v[:],
                                            in1=h0s[:, g, :], op=Alu.add)
                    nc.vector.tensor_tensor(out=xcur[:, g, :],
                                            in0=xcur[:, g, :], in1=w[:],
                                            op=Alu.add)
                if l is not None:
                    # transpose xcur[g], z = xcur @ Wc[l] -> z_all + zbounce
                    tp = psum_t.tile([128, 2, 128], f32, name="tp")
                    xcurT = small.tile([128, 2, 128], bf16, name="xcurT")
                    for kt in range(2):
                        nc.tensor.transpose(
                            out=tp[:, kt, :],
                            in_=xcur[:, g, kt * 128:(kt + 1) * 128],
                            identity=ident[:])
                        nc.scalar.activation(
                            out=xcurT[:, kt, :], in_=tp[:, kt, :],
                            func=mybir.ActivationFunctionType.Copy)
                    zp = psum_z.tile([128, H], f32, name="zp")
                    for kt in range(2):
                        nc.tensor.matmul(
                            out=zp[:], lhsT=xcurT[:, kt, :],
                            rhs=Wc_sb[:, l * 2 + kt, :],
                            start=(kt == 0), stop=(kt == 1))
                    nc.scalar.activation(
                        out=z_all[:, g, :], in_=zp[:],
                        func=mybir.ActivationFunctionType.Copy)
                    nc.sync.dma_start(
                        out=zbounces[l].ap()[g * 128:(g + 1) * 128, :],
                        in_=z_all[:, g, :])
                else:
                    nc.sync.dma_start(
                        out=out_dram.ap()[g * 128:(g + 1) * 128, :],
                        in_=xcur[:, g, :])

            # clear msgs buffers once: -1 pad slots are never written by the
            # gather, and S weights of 0 must multiply finite values
            nc.vector.memset(msgs_all[:], 0.0)

            # ---------- input block ----------
            for g in range(G):
                hp = psum_a.tile([128, H], f32, name="agg")
                nc.tensor.matmul(out=hp[:],
                                 lhsT=xT_sb[:, g * 128:(g + 1) * 128],
                                 rhs=W_in_sb[:], start=True, stop=False)
                nc.tensor.matmul(out=hp[:], lhsT=ones_sb[:],
                                 rhs=bias_sb[:, 0, :], start=False,
                                 stop=True)
                gelu_stats(g, hp)
            batched_rstd()
            for g in range(G):
                ln_blend_z(g, y_all[:, g, :], mv_all[:, g, :],
                           rstd_all[:, g:g + 1], 0, 1, 0, first=True)

            for l in range(L):
                nc.gpsimd.collective_compute(
                    "AllGather", mybir.AluOpType.bypass,
                    replica_groups=[list(range(NCORES))],
                    ins=[zbounces[l].ap()], outs=[zfulls[l].ap()])

                for b in range(NB):
                    buf = (l * NB + b) % MB
                    msgs = msgs_all[:, buf, :, :]
                    nc.gpsimd.dma_gather(
                        msgs, zfulls[l].ap(),
                        gidx_sb[:, b * BLK:(b + 1) * BLK, :],
                        num_idxs=BLK * SLOTS, num_idxs_reg=BLK * SLOTS,
                        elem_size=H, single_packet=False)
                    for gsub in range(BLK):
                        g = b * BLK + gsub
                        s_sb = s_pool.tile([128, C, 128], bf16, name="s_sb")
                        nc.sync.dma_start(out=s_sb[:], in_=S_in.ap()[g])
                        agg = psum_a.tile([128, H], f32, name="agg")
                        # self-loop diagonal first
                        nc.tensor.matmul(out=agg[:], lhsT=D_sb[:, g, :],
                                         rhs=z_all[:, g, :], start=True,
                                         stop=False)
                        for c in range(C):
                            nc.tensor.matmul(
                                out=agg[:], lhsT=s_sb[:, c, :],
                                rhs=msgs[:, gsub * C + c, :],
                                start=False, stop=False)
                        nc.tensor.matmul(
                            out=agg[:], lhsT=ones_sb[:],
                            rhs=bias_sb[:, 1 + l, :],
                            start=False, stop=True)
                        gelu_stats(g, agg)
                batched_rstd()
                for g in range(G):
                    ln_blend_z(g, y_all[:, g, :], mv_all[:, g, :],
                               rstd_all[:, g:g + 1], 2 + l, 2 + L + l,
                               l + 1 if l < L - 1 else None, first=False)

    nc.compile()
    return nc


def _get_program(cfg, C):
    key = (cfg, C)
    if key not in _cache:
        _cache[key] = _build(cfg, C)
    return _cache[key]


def run_sharded(inputs, trace=False, cfg=DEFAULT_CFG):
    in_maps, C = _preprocess(cfg, **inputs)
    nc = _get_program(cfg, C)
    res = bass_utils.run_bass_kernel_spmd(
        nc, in_maps, core_ids=list(range(NCORES)), trace=trace)
    out = np.empty((cfg.n, H), dtype=np.float32)
    for c in range(NCORES):
        out[c * cfg.percore:(c + 1) * cfg.percore] = \
            res.results[c]["out"][:cfg.percore]
    return out, res


def kernel(**inputs):
    out, _ = run_sharded(inputs, trace=False)
    return out


# revision 35
# speedup vs baseline: 1.5230x; 1.2448x over previous
"""GCN encoder kernel for Trainium2 (8 NeuronCores).

Strategy (graph/data parallel, per sharding hint):
  - Nodes sharded by destination range across 8 cores (2500 -> padded 2560/core).
  - Host precomputes GCN symmetric normalization and a per-destination-group
    schedule: non-self edges are deduplicated by source per 128-dst group.
    The segment-sum becomes dense matmuls with tiny scatter matrices
    S[src_slot, dst] holding the summed edge norms; self-loops are applied as
    a diagonal matmul against the locally-kept z.
  - Per layer: AllGather z (bf16) across cores, dma_gather the unique source
    rows (2 groups per instruction), TensorE matmuls accumulate messages into
    PSUM per dst group (per-layer bias folded in as a rank-1 matmul), then
    GELU + LayerNorm (rstd batched per layer: one ACT Sqrt + DVE reciprocal,
    so only two ACT table loads per layer) + residual blend.
"""

import sys
from dataclasses import dataclass

sys.path.insert(0, "/opt/trn_rl_repo")

import numpy as np
import ml_dtypes

import concourse.bass as bass
import concourse.tile as tile
from concourse import bacc, mybir
from concourse import bass_utils
from concourse.masks import make_identity

BF16 = ml_dtypes.bfloat16
ALPHA = 0.1
EPS = 1e-5
IN_DIM = 128
H = 256
NCORES = 8
ACT_FN = mybir.ActivationFunctionType.Gelu_apprx_tanh
QUAKE_MAGIC = 0x5F3759DF


@dataclass(frozen=True)
class Cfg:
    n: int = 20000
    layers: int = 6

    @property
    def percore(self):
        return self.n // NCORES

    @property
    def groups(self):
        return (self.percore + 127) // 128

    @property
    def padn(self):
        return self.groups * 128


DEFAULT_CFG = Cfg()
_cache = {}


def _preprocess(cfg, x, edge_index, W_in, b_in, g_in, beta_in, Wc, bc, gc,
                betac):
    """Host-side graph preprocessing -> per-core input maps."""
    N, G, PERCORE, PADN, L = (cfg.n, cfg.groups, cfg.percore, cfg.padn,
                              cfg.layers)
    x = np.asarray(x, dtype=np.float32)
    ei = np.asarray(edge_index).astype(np.int64)
    W_in = np.asarray(W_in, dtype=np.float32)
    b_in = np.asarray(b_in, dtype=np.float32)
    g_in = np.asarray(g_in, dtype=np.float32)
    beta_in = np.asarray(beta_in, dtype=np.float32)
    Wc = np.asarray(Wc, dtype=np.float32)
    bc = np.asarray(bc, dtype=np.float32)
    gc = np.asarray(gc, dtype=np.float32)
    betac = np.asarray(betac, dtype=np.float32)

    loop = np.arange(N, dtype=np.int64)
    col_all = np.concatenate([ei[1], loop])   # dst (for degree)
    deg = np.bincount(col_all, minlength=N).astype(np.float32)
    dinv = np.where(deg > 0, 1.0 / np.sqrt(deg), 0.0).astype(np.float32)

    # non-self edges (self loops handled by the diagonal matmul)
    row = ei[0]
    col = ei[1]
    norm = (dinv[row] * dinv[col]).astype(np.float32)
    pad_src = (row // PERCORE) * PADN + (row % PERCORE)

    core_of = col // PERCORE
    dloc_all = col % PERCORE
    grp_all = dloc_all // 128
    dst_all = dloc_all % 128

    per_core = []
    maxU = 1
    for c in range(NCORES):
        m = core_of == c
        g_e = grp_all[m]
        d_e = dst_all[m]
        s_e = pad_src[m]
        n_e = norm[m]
        # dedup sources within each group
        key = g_e * (NCORES * PADN) + s_e
        ukey, slot_of_edge = np.unique(key, return_inverse=True)
        u_grp = ukey // (NCORES * PADN)
        u_src = ukey % (NCORES * PADN)
        counts = np.bincount(u_grp, minlength=G)
        maxU = max(maxU, counts.max())
        per_core.append((g_e, d_e, s_e, n_e, slot_of_edge, u_grp, u_src,
                         counts))

    C = int((maxU + 127) // 128)
    SLOTS = C * 128
    S16 = SLOTS // 16

    in_maps = []
    for c in range(NCORES):
        g_e, d_e, s_e, n_e, slot_of_edge, u_grp, u_src, counts = per_core[c]
        starts = np.zeros(G + 1, dtype=np.int64)
        np.cumsum(counts, out=starts[1:])
        # slot of each unique (group, src) within its group
        u_slot = np.arange(len(u_grp)) - starts[u_grp]

        # Pads are dummy index 0 (gathered but weighted 0 in S): groups are
        # batched 4-per-gather-instruction, so pads sit interior where the
        # ucode's trailing -1 trim cannot apply.
        idx_l = np.zeros((G, SLOTS), dtype=np.int16)
        idx_l[u_grp, u_slot] = u_src.astype(np.int16)

        S_l = np.zeros((G, SLOTS, 128), dtype=np.float32)
        np.add.at(S_l, (g_e, u_slot[slot_of_edge], d_e), n_e)

        # idx layout: logical slot i -> partition 16*stripe + i%16, col i//16
        idx_rs = idx_l.reshape(G, S16, 16).transpose(2, 0, 1)   # [16, G, S16]
        gidx = np.tile(idx_rs, (8, 1, 1)).astype(np.int16)      # [128, G, S16]

        # S layout: [G, 128(slot%128), C(chunk), 128(dst)]
        S_arr = S_l.reshape(G, C, 128, 128).transpose(0, 2, 1, 3).astype(BF16)

        # self-loop diagonal: D[g, p, p] = dinv[node]^2
        nodes = np.arange(PERCORE) + c * PERCORE
        d2 = np.zeros(PADN, dtype=np.float32)
        d2[:PERCORE] = dinv[nodes] ** 2
        D_arr = np.zeros((G, 128, 128), dtype=np.float32)
        pi = np.arange(128)
        for g in range(G):
            D_arr[g, pi, pi] = d2[g * 128:(g + 1) * 128]
        D_arr = D_arr.astype(BF16)

        xs = x[c * PERCORE:(c + 1) * PERCORE]
        xT = np.zeros((IN_DIM, PADN), dtype=BF16)
        xT[:, :PERCORE] = xs.T.astype(BF16)

        in_maps.append({
            "xT": np.ascontiguousarray(xT),
            "gidx": np.ascontiguousarray(gidx),
            "S": np.ascontiguousarray(S_arr),
            "D": np.ascontiguousarray(D_arr),
        })

    Wc_bf = np.ascontiguousarray(Wc.reshape(L, 2, 128, H).astype(BF16))
    W_in_bf = W_in.astype(BF16)
    biases = np.concatenate([b_in[None, :], bc], axis=0).astype(BF16)
    ln = np.zeros((2 + 2 * L, H), dtype=np.float32)
    ln[0] = g_in
    ln[1] = beta_in
    ln[2:2 + L] = (1.0 - ALPHA) * gc
    ln[2 + L:2 + 2 * L] = (1.0 - ALPHA) * betac
    for m in in_maps:
        m["Wc"] = Wc_bf
        m["W_in"] = W_in_bf
        m["biases"] = biases
        m["ln"] = ln

    return in_maps, C


def _build(cfg, C):
    """Build the Bass program (shared by all 8 cores)."""
    G, PADN, L = cfg.groups, cfg.padn, cfg.layers
    SLOTS = C * 128
    S16 = SLOTS // 16
    f32 = mybir.dt.float32
    i32 = mybir.dt.int32
    bf16 = mybir.dt.bfloat16
    Alu = mybir.AluOpType

    nc = bacc.Bacc("TRN2", target_bir_lowering=False, debug=False,
                   num_devices=NCORES, num_swdge_queues=4)

    xT_in = nc.dram_tensor("xT", [IN_DIM, PADN], bf16, kind="ExternalInput")
    gidx_in = nc.dram_tensor("gidx", [128, G, S16], mybir.dt.int16,
                             kind="ExternalInput")
    S_in = nc.dram_tensor("S", [G, 128, C, 128], bf16, kind="ExternalInput")
    D_in = nc.dram_tensor("D", [G, 128, 128], bf16, kind="ExternalInput")
    Wc_in = nc.dram_tensor("Wc", [L, 2, 128, H], bf16, kind="ExternalInput")
    W_in_in = nc.dram_tensor("W_in", [IN_DIM, H], bf16, kind="ExternalInput")
    biases_in = nc.dram_tensor("biases", [L + 1, H], bf16,
                               kind="ExternalInput")
    ln_in = nc.dram_tensor("ln", [2 + 2 * L, H], f32, kind="ExternalInput")
    out_dram = nc.dram_tensor("out", [PADN, H], f32, kind="ExternalOutput")

    zbounces = [nc.dram_tensor(f"zbounce{l}", [PADN, H], bf16,
                               kind="Internal") for l in range(L)]
    # Shared output -> one-hop peer writes instead of RDH hops
    zfulls = [nc.dram_tensor(f"zfull{l}", [NCORES * PADN, H], bf16,
                             kind="Internal", addr_space="Shared")
              for l in range(L)]

    def bcast128(ap_row):
        return bass.AP(tensor=ap_row.tensor, offset=ap_row.offset,
                       ap=[[0, 128]] + list(ap_row.ap[1:]))

    with tile.TileContext(nc) as tc:
        with (
            tc.tile_pool(name="persist", bufs=1) as pp,
            tc.tile_pool(name="msgs_pool", bufs=4) as msgs_pool,
            tc.tile_pool(name="s_pool", bufs=6) as s_pool,
            tc.tile_pool(name="small", bufs=4) as small,
            tc.tile_pool(name="tiny", bufs=6) as tiny,
            tc.tile_pool(name="psum_a", bufs=3, space="PSUM") as psum_a,
            tc.tile_pool(name="psum_z", bufs=2, space="PSUM") as psum_z,
            tc.tile_pool(name="psum_t", bufs=2, space="PSUM") as psum_t,
        ):
            # ---------- persistent tiles ----------
            xcur = pp.tile([128, G, H], f32)
            h0s = pp.tile([128, G, H], f32)
            z_all = pp.tile([128, G, H], bf16)
            BLK = 2 if G % 2 == 0 else 1
            NB = G // BLK
            MB = 4
            msgs_all = pp.tile([128, MB, BLK * C, H], bf16)
            y_all = pp.tile([128, G, H], bf16)
            mv_all = pp.tile([128, G, 2], f32)
            rstd_all = pp.tile([128, G], f32)
            eps_sb = pp.tile([128, 1], f32)
            nc.vector.memset(eps_sb[:], EPS)
            gidx_sb = pp.tile([128, G, S16], mybir.dt.int16)
            D_sb = pp.tile([128, G, 128], bf16)
            Wc_sb = pp.tile([128, L * 2, H], bf16)
            W_in_sb = pp.tile([128, H], bf16)
            bias_sb = pp.tile([1, L + 1, H], bf16)
            ones_sb = pp.tile([1, 128], bf16)
            ln_sb = pp.tile([128, 2 + 2 * L, H], f32)
            ident = pp.tile([128, 128], f32)
            xT_sb = pp.tile([128, PADN], bf16)

            nc.sync.dma_start(out=gidx_sb[:], in_=gidx_in.ap())
            for g in range(G):
                nc.sync.dma_start(out=D_sb[:, g, :], in_=D_in.ap()[g])
            for l in range(L):
                for kt in range(2):
                    nc.sync.dma_start(out=Wc_sb[:, l * 2 + kt, :],
                                      in_=Wc_in.ap()[l, kt])
            nc.sync.dma_start(out=W_in_sb[:], in_=W_in_in.ap())
            nc.sync.dma_start(out=bias_sb[:], in_=biases_in.ap()[None])
            nc.vector.memset(ones_sb[:], 1.0)
            for r in range(2 + 2 * L):
                nc.sync.dma_start(out=ln_sb[:, r, :],
                                  in_=bcast128(ln_in.ap()[r:r + 1, :]))
            make_identity(nc, ident[:])
            nc.sync.dma_start(out=xT_sb[:], in_=xT_in.ap())

            def batched_rstd():
                nc.scalar.activation(
                    out=rstd_all[:], in_=mv_all[:, :, 1],
                    func=mybir.ActivationFunctionType.Sqrt, bias=eps_sb[:])
                nc.vector.reciprocal(out=rstd_all[:], in_=rstd_all[:])

            def gelu_stats(g, psum):
                nc.scalar.activation(out=y_all[:, g, :], in_=psum[:],
                                     func=ACT_FN)
                stats = tiny.tile([128, 6], f32, name="bn_st")
                nc.vector.bn_stats(out=stats[:], in_=y_all[:, g, :])
                nc.vector.bn_aggr(out=mv_all[:, g, :], in_=stats[:])

            def ln_blend_z(g, y_ap, mv_ap, rstd_ap, gi, bi, l, first):
                """LN + blend for group g, then z for layer l."""
                t = small.tile([128, H], f32, name="t_ln")
                nc.vector.tensor_scalar_sub(out=t[:], in0=y_ap,
                                            scalar1=mv_ap[0:128, 0:1])
                u = small.tile([128, H], f32, name="u_ln")
                nc.vector.scalar_tensor_tensor(
                    out=u[:], in0=t[:], scalar=rstd_ap, in1=ln_sb[:, gi, :],
                    op0=Alu.mult, op1=Alu.mult)
                if first:
                    nc.vector.tensor_tensor(out=xcur[:, g, :], in0=u[:],
                                            in1=ln_sb[:, bi, :], op=Alu.add)
                    nc.vector.tensor_scalar_mul(out=h0s[:, g, :],
                                                in0=xcur[:, g, :],
                                                scalar1=ALPHA)
                else:
                    v = small.tile([128, H], f32, name="v_ln")
                    nc.vector.tensor_tensor(out=v[:], in0=u[:],
                                            in1=ln_sb[:, bi, :], op=Alu.add)
                    w = small.tile([128, H], f32, name="w_ln")
                    nc.vector.tensor_tensor(out=w[:], in0=v[:],
                                            in1=h0s[:, g, :], op=Alu.add)
                    nc.vector.tensor_tensor(out=xcur[:, g, :],
                                            in0=xcur[:, g, :], in1=w[:],
                                            op=Alu.add)
                if l is not None:
                    # transpose xcur[g], z = xcur @ Wc[l] -> z_all + zbounce
                    tp = psum_t.tile([128, 2, 128], f32, name="tp")
                    xcurT = small.tile([128, 2, 128], bf16, name="xcurT")
                    for kt in range(2):
                        nc.tensor.transpose(
                            out=tp[:, kt, :],
                            in_=xcur[:, g, kt * 128:(kt + 1) * 128],
                            identity=ident[:])
                        nc.scalar.activation(
                            out=xcurT[:, kt, :], in_=tp[:, kt, :],
                            func=mybir.ActivationFunctionType.Copy)
                    zp = psum_z.tile([128, H], f32, name="zp")
                    for kt in range(2):
                        nc.tensor.matmul(
                            out=zp[:], lhsT=xcurT[:, kt, :],
                            rhs=Wc_sb[:, l * 2 + kt, :],
                            start=(kt == 0), stop=(kt == 1))
                    nc.scalar.activation(
                        out=z_all[:, g, :], in_=zp[:],
                        func=mybir.ActivationFunctionType.Copy)
                    nc.sync.dma_start(
                        out=zbounces[l].ap()[g * 128:(g + 1) * 128, :],
                        in_=z_all[:, g, :])
                else:
                    nc.sync.dma_start(
                        out=out_dram.ap()[g * 128:(g + 1) * 128, :],
                        in_=xcur[:, g, :])

            # clear msgs buffers once: -1 pad slots are never written by the
            # gather, and S weights of 0 must multiply finite values
            nc.vector.memset(msgs_all[:], 0.0)

            # ---------- input block ----------
            for g in range(G):
                hp = psum_a.tile([128, H], f32, name="agg")
                nc.tensor.matmul(out=hp[:],
                                 lhsT=xT_sb[:, g * 128:(g + 1) * 128],
                                 rhs=W_in_sb[:], start=True, stop=False)
                nc.tensor.matmul(out=hp[:], lhsT=ones_sb[:],
                                 rhs=bias_sb[:, 0, :], start=False,
                                 stop=True)
                gelu_stats(g, hp)
            batched_rstd()
            for g in range(G):
                ln_blend_z(g, y_all[:, g, :], mv_all[:, g, :],
                           rstd_all[:, g:g + 1], 0, 1, 0, first=True)

            for l in range(L):
                nc.gpsimd.collective_compute(
                    "AllGather", mybir.AluOpType.bypass,
                    replica_groups=[list(range(NCORES))],
                    ins=[zbounces[l].ap()], outs=[zfulls[l].ap()])

                for b in range(NB):
                    buf = (l * NB + b) % MB
                    msgs = msgs_all[:, buf, :, :]
                    nc.gpsimd.dma_gather(
                        msgs, zfulls[l].ap(),
                        gidx_sb[:, b * BLK:(b + 1) * BLK, :],
                        num_idxs=BLK * SLOTS, num_idxs_reg=BLK * SLOTS,
                        elem_size=H, single_packet=False,
                        queue_num=(l * NB + b) % 4)
                    for gsub in range(BLK):
                        g = b * BLK + gsub
                        s_sb = s_pool.tile([128, C, 128], bf16, name="s_sb")
                        nc.sync.dma_start(out=s_sb[:], in_=S_in.ap()[g])
                        agg = psum_a.tile([128, H], f32, name="agg")
                        # self-loop diagonal first
                        nc.tensor.matmul(out=agg[:], lhsT=D_sb[:, g, :],
                                         rhs=z_all[:, g, :], start=True,
                                         stop=False)
                        for c in range(C):
                            nc.tensor.matmul(
                                out=agg[:], lhsT=s_sb[:, c, :],
                                rhs=msgs[:, gsub * C + c, :],
                                start=False, stop=False)
                        nc.tensor.matmul(
                            out=agg[:], lhsT=ones_sb[:],
                            rhs=bias_sb[:, 1 + l, :],
                            start=False, stop=True)
                        gelu_stats(g, agg)
                batched_rstd()
                for g in range(G):
                    ln_blend_z(g, y_all[:, g, :], mv_all[:, g, :],
                               rstd_all[:, g:g + 1], 2 + l, 2 + L + l,
                               l + 1 if l < L - 1 else None, first=False)

    nc.compile()
    return nc


def _get_program(cfg, C):
    key = (cfg, C)
    if key not in _cache:
        _cache[key] = _build(cfg, C)
    return _cache[key]


def run_sharded(inputs, trace=False, cfg=DEFAULT_CFG):
    in_maps, C = _preprocess(cfg, **inputs)
    nc = _get_program(cfg, C)
    res = bass_utils.run_bass_kernel_spmd(
        nc, in_maps, core_ids=list(range(NCORES)), trace=trace)
    out = np.empty((cfg.n, H), dtype=np.float32)
    for c in range(NCORES):
        out[c * cfg.percore:(c + 1) * cfg.percore] = \
            res.results[c]["out"][:cfg.percore]
    return out, res


def kernel(**inputs):
    out, _ = run_sharded(inputs, trace=False)
    return out


# revision 37
# speedup vs baseline: 1.6130x; 1.0591x over previous
"""GCN encoder kernel for Trainium2 (8 NeuronCores).

Strategy (graph/data parallel, per sharding hint):
  - Nodes sharded by destination range across 8 cores (2500 -> padded 2560/core).
  - Host precomputes GCN symmetric normalization and a per-destination-group
    schedule: non-self edges are deduplicated by source per 128-dst group.
    The segment-sum becomes dense matmuls with tiny scatter matrices
    S[src_slot, dst] holding the summed edge norms; self-loops are applied as
    a diagonal matmul against the locally-kept z.
  - Per layer: AllGather z (bf16) across cores, dma_gather the unique source
    rows (2 groups per instruction), TensorE matmuls accumulate messages into
    PSUM per dst group (per-layer bias folded in as a rank-1 matmul), then
    GELU + LayerNorm (rstd batched per layer: one ACT Sqrt + DVE reciprocal,
    so only two ACT table loads per layer) + residual blend.
"""

import sys
from dataclasses import dataclass

sys.path.insert(0, "/opt/trn_rl_repo")

import numpy as np
import ml_dtypes

import concourse.bass as bass
import concourse.tile as tile
from concourse import bacc, mybir
from concourse import bass_utils
from concourse.masks import make_identity

BF16 = ml_dtypes.bfloat16
ALPHA = 0.1
EPS = 1e-5
IN_DIM = 128
H = 256
NCORES = 8
ACT_FN = mybir.ActivationFunctionType.Gelu_apprx_tanh
QUAKE_MAGIC = 0x5F3759DF


@dataclass(frozen=True)
class Cfg:
    n: int = 20000
    layers: int = 6

    @property
    def percore(self):
        return self.n // NCORES

    @property
    def groups(self):
        return (self.percore + 127) // 128

    @property
    def padn(self):
        return self.groups * 128


DEFAULT_CFG = Cfg()
_cache = {}


def _preprocess(cfg, x, edge_index, W_in, b_in, g_in, beta_in, Wc, bc, gc,
                betac):
    """Host-side graph preprocessing -> per-core input maps."""
    N, G, PERCORE, PADN, L = (cfg.n, cfg.groups, cfg.percore, cfg.padn,
                              cfg.layers)
    x = np.asarray(x, dtype=np.float32)
    ei = np.asarray(edge_index).astype(np.int64)
    W_in = np.asarray(W_in, dtype=np.float32)
    b_in = np.asarray(b_in, dtype=np.float32)
    g_in = np.asarray(g_in, dtype=np.float32)
    beta_in = np.asarray(beta_in, dtype=np.float32)
    Wc = np.asarray(Wc, dtype=np.float32)
    bc = np.asarray(bc, dtype=np.float32)
    gc = np.asarray(gc, dtype=np.float32)
    betac = np.asarray(betac, dtype=np.float32)

    loop = np.arange(N, dtype=np.int64)
    col_all = np.concatenate([ei[1], loop])   # dst (for degree)
    deg = np.bincount(col_all, minlength=N).astype(np.float32)
    dinv = np.where(deg > 0, 1.0 / np.sqrt(deg), 0.0).astype(np.float32)

    # non-self edges (self loops handled by the diagonal matmul)
    row = ei[0]
    col = ei[1]
    norm = (dinv[row] * dinv[col]).astype(np.float32)
    pad_src = (row // PERCORE) * PADN + (row % PERCORE)

    core_of = col // PERCORE
    dloc_all = col % PERCORE
    grp_all = dloc_all // 128
    dst_all = dloc_all % 128

    per_core = []
    maxU = 1
    for c in range(NCORES):
        m = core_of == c
        g_e = grp_all[m]
        d_e = dst_all[m]
        s_e = pad_src[m]
        n_e = norm[m]
        # dedup sources within each group
        key = g_e * (NCORES * PADN) + s_e
        ukey, slot_of_edge = np.unique(key, return_inverse=True)
        u_grp = ukey // (NCORES * PADN)
        u_src = ukey % (NCORES * PADN)
        counts = np.bincount(u_grp, minlength=G)
        maxU = max(maxU, counts.max())
        per_core.append((g_e, d_e, s_e, n_e, slot_of_edge, u_grp, u_src,
                         counts))

    C = int((maxU + 127) // 128)
    SLOTS = C * 128
    S16 = SLOTS // 16

    in_maps = []
    for c in range(NCORES):
        g_e, d_e, s_e, n_e, slot_of_edge, u_grp, u_src, counts = per_core[c]
        starts = np.zeros(G + 1, dtype=np.int64)
        np.cumsum(counts, out=starts[1:])
        # slot of each unique (group, src) within its group
        u_slot = np.arange(len(u_grp)) - starts[u_grp]

        # Pads are dummy index 0 (gathered but weighted 0 in S): groups are
        # batched 4-per-gather-instruction, so pads sit interior where the
        # ucode's trailing -1 trim cannot apply.
        idx_l = np.zeros((G, SLOTS), dtype=np.int16)
        idx_l[u_grp, u_slot] = u_src.astype(np.int16)

        S_l = np.zeros((G, SLOTS, 128), dtype=np.float32)
        np.add.at(S_l, (g_e, u_slot[slot_of_edge], d_e), n_e)

        # idx layout: logical slot i -> partition 16*stripe + i%16, col i//16
        idx_rs = idx_l.reshape(G, S16, 16).transpose(2, 0, 1)   # [16, G, S16]
        gidx = np.tile(idx_rs, (8, 1, 1)).astype(np.int16)      # [128, G, S16]

        # S layout: [G, 128(slot%128), C(chunk), 128(dst)]
        S_arr = S_l.reshape(G, C, 128, 128).transpose(0, 2, 1, 3).astype(BF16)

        # self-loop diagonal: D[g, p, p] = dinv[node]^2
        nodes = np.arange(PERCORE) + c * PERCORE
        d2 = np.zeros(PADN, dtype=np.float32)
        d2[:PERCORE] = dinv[nodes] ** 2
        D_arr = np.zeros((G, 128, 128), dtype=np.float32)
        pi = np.arange(128)
        for g in range(G):
            D_arr[g, pi, pi] = d2[g * 128:(g + 1) * 128]
        D_arr = D_arr.astype(BF16)

        xs = x[c * PERCORE:(c + 1) * PERCORE]
        xT = np.zeros((IN_DIM, PADN), dtype=BF16)
        xT[:, :PERCORE] = xs.T.astype(BF16)

        in_maps.append({
            "xT": np.ascontiguousarray(xT),
            "gidx": np.ascontiguousarray(gidx),
            "S": np.ascontiguousarray(S_arr),
            "D": np.ascontiguousarray(D_arr),
        })

    Wc_bf = np.ascontiguousarray(Wc.reshape(L, 2, 128, H).astype(BF16))
    W_in_bf = W_in.astype(BF16)
    biases = np.concatenate([b_in[None, :], bc], axis=0).astype(BF16)
    ln = np.zeros((2 + 2 * L, H), dtype=np.float32)
    ln[0] = g_in
    ln[1] = beta_in
    ln[2:2 + L] = (1.0 - ALPHA) * gc
    ln[2 + L:2 + 2 * L] = (1.0 - ALPHA) * betac
    for m in in_maps:
        m["Wc"] = Wc_bf
        m["W_in"] = W_in_bf
        m["biases"] = biases
        m["ln"] = ln

    return in_maps, C


def _build(cfg, C):
    """Build the Bass program (shared by all 8 cores)."""
    G, PADN, L = cfg.groups, cfg.padn, cfg.layers
    SLOTS = C * 128
    S16 = SLOTS // 16
    f32 = mybir.dt.float32
    i32 = mybir.dt.int32
    bf16 = mybir.dt.bfloat16
    Alu = mybir.AluOpType

    nc = bacc.Bacc("TRN2", target_bir_lowering=False, debug=False,
                   num_devices=NCORES, num_swdge_queues=4)

    xT_in = nc.dram_tensor("xT", [IN_DIM, PADN], bf16, kind="ExternalInput")
    gidx_in = nc.dram_tensor("gidx", [128, G, S16], mybir.dt.int16,
                             kind="ExternalInput")
    S_in = nc.dram_tensor("S", [G, 128, C, 128], bf16, kind="ExternalInput")
    D_in = nc.dram_tensor("D", [G, 128, 128], bf16, kind="ExternalInput")
    Wc_in = nc.dram_tensor("Wc", [L, 2, 128, H], bf16, kind="ExternalInput")
    W_in_in = nc.dram_tensor("W_in", [IN_DIM, H], bf16, kind="ExternalInput")
    biases_in = nc.dram_tensor("biases", [L + 1, H], bf16,
                               kind="ExternalInput")
    ln_in = nc.dram_tensor("ln", [2 + 2 * L, H], f32, kind="ExternalInput")
    out_dram = nc.dram_tensor("out", [PADN, H], f32, kind="ExternalOutput")

    zbounces = [nc.dram_tensor(f"zbounce{l}", [PADN, H], bf16,
                               kind="Internal") for l in range(L)]
    # Shared output -> one-hop peer writes instead of RDH hops
    zfulls = [nc.dram_tensor(f"zfull{l}", [NCORES * PADN, H], bf16,
                             kind="Internal", addr_space="Shared")
              for l in range(L)]

    def bcast128(ap_row):
        return bass.AP(tensor=ap_row.tensor, offset=ap_row.offset,
                       ap=[[0, 128]] + list(ap_row.ap[1:]))

    with tile.TileContext(nc) as tc:
        with (
            tc.tile_pool(name="persist", bufs=1) as pp,
            tc.tile_pool(name="msgs_pool", bufs=4) as msgs_pool,
            tc.tile_pool(name="s_pool", bufs=6) as s_pool,
            tc.tile_pool(name="small", bufs=4) as small,
            tc.tile_pool(name="tiny", bufs=6) as tiny,
            tc.tile_pool(name="psum_a", bufs=3, space="PSUM") as psum_a,
            tc.tile_pool(name="psum_z", bufs=2, space="PSUM") as psum_z,
            tc.tile_pool(name="psum_t", bufs=2, space="PSUM") as psum_t,
        ):
            # ---------- persistent tiles ----------
            xcur = pp.tile([128, G, H], f32)
            h0s = pp.tile([128, G, H], f32)
            z_all = pp.tile([128, G, H], bf16)
            BLK = 2 if G % 2 == 0 else 1
            NB = G // BLK
            MB = 4
            msgs_all = pp.tile([128, MB, BLK * C, H], bf16)
            y_all = pp.tile([128, G, H], bf16)
            mv_all = pp.tile([128, G, 2], f32)
            rstd_all = pp.tile([128, G], f32)
            eps_sb = pp.tile([128, 1], f32)
            nc.vector.memset(eps_sb[:], EPS)
            gidx_sb = pp.tile([128, G, S16], mybir.dt.int16)
            D_sb = pp.tile([128, G, 128], bf16)
            Wc_sb = pp.tile([128, L * 2, H], bf16)
            W_in_sb = pp.tile([128, H], bf16)
            bias_sb = pp.tile([1, L + 1, H], bf16)
            ones_sb = pp.tile([1, 128], bf16)
            ln_sb = pp.tile([128, 2 + 2 * L, H], f32)
            ident = pp.tile([128, 128], f32)
            xT_sb = pp.tile([128, PADN], bf16)

            nc.sync.dma_start(out=gidx_sb[:], in_=gidx_in.ap())
            for g in range(G):
                nc.sync.dma_start(out=D_sb[:, g, :], in_=D_in.ap()[g])
            for l in range(L):
                for kt in range(2):
                    nc.sync.dma_start(out=Wc_sb[:, l * 2 + kt, :],
                                      in_=Wc_in.ap()[l, kt])
            nc.sync.dma_start(out=W_in_sb[:], in_=W_in_in.ap())
            nc.sync.dma_start(out=bias_sb[:], in_=biases_in.ap()[None])
            nc.vector.memset(ones_sb[:], 1.0)
            for r in range(2 + 2 * L):
                nc.sync.dma_start(out=ln_sb[:, r, :],
                                  in_=bcast128(ln_in.ap()[r:r + 1, :]))
            make_identity(nc, ident[:])
            nc.sync.dma_start(out=xT_sb[:], in_=xT_in.ap())

            def batched_rstd(g0=0, g1=G):
                nc.scalar.activation(
                    out=rstd_all[:, g0:g1], in_=mv_all[:, g0:g1, 1],
                    func=mybir.ActivationFunctionType.Sqrt, bias=eps_sb[:])
                nc.vector.reciprocal(out=rstd_all[:, g0:g1],
                                     in_=rstd_all[:, g0:g1])

            def gelu_stats(g, psum):
                nc.scalar.activation(out=y_all[:, g, :], in_=psum[:],
                                     func=ACT_FN)
                stats = tiny.tile([128, 6], f32, name="bn_st")
                nc.vector.bn_stats(out=stats[:], in_=y_all[:, g, :])
                nc.vector.bn_aggr(out=mv_all[:, g, :], in_=stats[:])

            def ln_blend_z(g, y_ap, mv_ap, rstd_ap, gi, bi, l, first):
                """LN + blend for group g, then z for layer l."""
                t = small.tile([128, H], f32, name="t_ln")
                nc.vector.tensor_scalar_sub(out=t[:], in0=y_ap,
                                            scalar1=mv_ap[0:128, 0:1])
                u = small.tile([128, H], f32, name="u_ln")
                nc.vector.scalar_tensor_tensor(
                    out=u[:], in0=t[:], scalar=rstd_ap, in1=ln_sb[:, gi, :],
                    op0=Alu.mult, op1=Alu.mult)
                if first:
                    nc.vector.tensor_tensor(out=xcur[:, g, :], in0=u[:],
                                            in1=ln_sb[:, bi, :], op=Alu.add)
                    nc.vector.tensor_scalar_mul(out=h0s[:, g, :],
                                                in0=xcur[:, g, :],
                                                scalar1=ALPHA)
                else:
                    v = small.tile([128, H], f32, name="v_ln")
                    nc.vector.tensor_tensor(out=v[:], in0=u[:],
                                            in1=ln_sb[:, bi, :], op=Alu.add)
                    w = small.tile([128, H], f32, name="w_ln")
                    nc.vector.tensor_tensor(out=w[:], in0=v[:],
                                            in1=h0s[:, g, :], op=Alu.add)
                    nc.vector.tensor_tensor(out=xcur[:, g, :],
                                            in0=xcur[:, g, :], in1=w[:],
                                            op=Alu.add)
                if l is not None:
                    # transpose xcur[g], z = xcur @ Wc[l] -> z_all + zbounce
                    tp = psum_t.tile([128, 2, 128], f32, name="tp")
                    xcurT = small.tile([128, 2, 128], bf16, name="xcurT")
                    for kt in range(2):
                        nc.tensor.transpose(
                            out=tp[:, kt, :],
                            in_=xcur[:, g, kt * 128:(kt + 1) * 128],
                            identity=ident[:])
                        nc.scalar.activation(
                            out=xcurT[:, kt, :], in_=tp[:, kt, :],
                            func=mybir.ActivationFunctionType.Copy)
                    zp = psum_z.tile([128, H], f32, name="zp")
                    for kt in range(2):
                        nc.tensor.matmul(
                            out=zp[:], lhsT=xcurT[:, kt, :],
                            rhs=Wc_sb[:, l * 2 + kt, :],
                            start=(kt == 0), stop=(kt == 1))
                    nc.scalar.activation(
                        out=z_all[:, g, :], in_=zp[:],
                        func=mybir.ActivationFunctionType.Copy)
                    nc.sync.dma_start(
                        out=zbounces[l].ap()[g * 128:(g + 1) * 128, :],
                        in_=z_all[:, g, :])
                else:
                    nc.sync.dma_start(
                        out=out_dram.ap()[g * 128:(g + 1) * 128, :],
                        in_=xcur[:, g, :])

            # clear msgs buffers once: -1 pad slots are never written by the
            # gather, and S weights of 0 must multiply finite values
            nc.vector.memset(msgs_all[:], 0.0)

            # ---------- input block ----------
            for g in range(G):
                hp = psum_a.tile([128, H], f32, name="agg")
                nc.tensor.matmul(out=hp[:],
                                 lhsT=xT_sb[:, g * 128:(g + 1) * 128],
                                 rhs=W_in_sb[:], start=True, stop=False)
                nc.tensor.matmul(out=hp[:], lhsT=ones_sb[:],
                                 rhs=bias_sb[:, 0, :], start=False,
                                 stop=True)
                gelu_stats(g, hp)
            batched_rstd()
            for g in range(G):
                ln_blend_z(g, y_all[:, g, :], mv_all[:, g, :],
                           rstd_all[:, g:g + 1], 0, 1, 0, first=True)

            for l in range(L):
                nc.gpsimd.collective_compute(
                    "AllGather", mybir.AluOpType.bypass,
                    replica_groups=[list(range(NCORES))],
                    ins=[zbounces[l].ap()], outs=[zfulls[l].ap()])

                # two half-layers: LN/blend of the first half overlaps the
                # gathers of the second half
                NBH = (NB + 1) // 2
                for half in range(2):
                    b_lo, b_hi = half * NBH, min((half + 1) * NBH, NB)
                    if b_lo >= b_hi:
                        continue
                    for b in range(b_lo, b_hi):
                        buf = (l * NB + b) % MB
                        msgs = msgs_all[:, buf, :, :]
                        nc.gpsimd.dma_gather(
                            msgs, zfulls[l].ap(),
                            gidx_sb[:, b * BLK:(b + 1) * BLK, :],
                            num_idxs=BLK * SLOTS, num_idxs_reg=BLK * SLOTS,
                            elem_size=H, single_packet=False,
                            queue_num=(l * NB + b) % 4)
                        for gsub in range(BLK):
                            g = b * BLK + gsub
                            s_sb = s_pool.tile([128, C, 128], bf16,
                                               name="s_sb")
                            nc.sync.dma_start(out=s_sb[:], in_=S_in.ap()[g])
                            agg = psum_a.tile([128, H], f32, name="agg")
                            # self-loop diagonal first
                            nc.tensor.matmul(out=agg[:], lhsT=D_sb[:, g, :],
                                             rhs=z_all[:, g, :], start=True,
                                             stop=False)
                            for c in range(C):
                                nc.tensor.matmul(
                                    out=agg[:], lhsT=s_sb[:, c, :],
                                    rhs=msgs[:, gsub * C + c, :],
                                    start=False, stop=False)
                            nc.tensor.matmul(
                                out=agg[:], lhsT=ones_sb[:],
                                rhs=bias_sb[:, 1 + l, :],
                                start=False, stop=True)
                            gelu_stats(g, agg)
                    batched_rstd(b_lo * BLK, b_hi * BLK)
                    for g in range(b_lo * BLK, b_hi * BLK):
                        ln_blend_z(g, y_all[:, g, :], mv_all[:, g, :],
                                   rstd_all[:, g:g + 1], 2 + l, 2 + L + l,
                                   l + 1 if l < L - 1 else None, first=False)

    nc.compile()
    return nc


def _get_program(cfg, C):
    key = (cfg, C)
    if key not in _cache:
        _cache[key] = _build(cfg, C)
    return _cache[key]


def run_sharded(inputs, trace=False, cfg=DEFAULT_CFG):
    in_maps, C = _preprocess(cfg, **inputs)
    nc = _get_program(cfg, C)
    res = bass_utils.run_bass_kernel_spmd(
        nc, in_maps, core_ids=list(range(NCORES)), trace=trace)
    out = np.empty((cfg.n, H), dtype=np.float32)
    for c in range(NCORES):
        out[c * cfg.percore:(c + 1) * cfg.percore] = \
            res.results[c]["out"][:cfg.percore]
    return out, res


def kernel(**inputs):
    out, _ = run_sharded(inputs, trace=False)
    return out


# revision 38
# speedup vs baseline: 1.8858x; 1.1691x over previous
"""GCN encoder kernel for Trainium2 (8 NeuronCores).

Strategy (graph/data parallel, per sharding hint):
  - Nodes sharded by destination range across 8 cores (2500 -> padded 2560/core).
  - Host precomputes GCN symmetric normalization and a per-destination-group
    schedule: non-self edges are deduplicated by source per 128-dst group.
    The segment-sum becomes dense matmuls with tiny scatter matrices
    S[src_slot, dst] holding the summed edge norms; self-loops are applied as
    a diagonal matmul against the locally-kept z.
  - Per layer: AllGather z (bf16) across cores, dma_gather the unique source
    rows (2 groups per instruction), TensorE matmuls accumulate messages into
    PSUM per dst group (per-layer bias folded in as a rank-1 matmul), then
    GELU + LayerNorm (rstd batched per layer: one ACT Sqrt + DVE reciprocal,
    so only two ACT table loads per layer) + residual blend.
"""

import sys
from dataclasses import dataclass

sys.path.insert(0, "/opt/trn_rl_repo")

import numpy as np
import ml_dtypes

import concourse.bass as bass
import concourse.tile as tile
from concourse import bacc, mybir
from concourse import bass_utils
from concourse.masks import make_identity

BF16 = ml_dtypes.bfloat16
ALPHA = 0.1
EPS = 1e-5
IN_DIM = 128
H = 256
NCORES = 8
ACT_FN = mybir.ActivationFunctionType.Gelu_apprx_tanh
QUAKE_MAGIC = 0x5F3759DF


@dataclass(frozen=True)
class Cfg:
    n: int = 20000
    layers: int = 6

    @property
    def percore(self):
        return self.n // NCORES

    @property
    def groups(self):
        return (self.percore + 127) // 128

    @property
    def padn(self):
        return self.groups * 128


DEFAULT_CFG = Cfg()
_cache = {}


def _preprocess(cfg, x, edge_index, W_in, b_in, g_in, beta_in, Wc, bc, gc,
                betac):
    """Host-side graph preprocessing -> per-core input maps."""
    N, G, PERCORE, PADN, L = (cfg.n, cfg.groups, cfg.percore, cfg.padn,
                              cfg.layers)
    x = np.asarray(x, dtype=np.float32)
    ei = np.asarray(edge_index).astype(np.int64)
    W_in = np.asarray(W_in, dtype=np.float32)
    b_in = np.asarray(b_in, dtype=np.float32)
    g_in = np.asarray(g_in, dtype=np.float32)
    beta_in = np.asarray(beta_in, dtype=np.float32)
    Wc = np.asarray(Wc, dtype=np.float32)
    bc = np.asarray(bc, dtype=np.float32)
    gc = np.asarray(gc, dtype=np.float32)
    betac = np.asarray(betac, dtype=np.float32)

    loop = np.arange(N, dtype=np.int64)
    col_all = np.concatenate([ei[1], loop])   # dst (for degree)
    deg = np.bincount(col_all, minlength=N).astype(np.float32)
    dinv = np.where(deg > 0, 1.0 / np.sqrt(deg), 0.0).astype(np.float32)

    # non-self edges (self loops handled by the diagonal matmul)
    row = ei[0]
    col = ei[1]
    norm = (dinv[row] * dinv[col]).astype(np.float32)
    pad_src = (row // PERCORE) * PADN + (row % PERCORE)

    core_of = col // PERCORE
    dloc_all = col % PERCORE
    grp_all = dloc_all // 128
    dst_all = dloc_all % 128

    per_core = []
    maxU = 1
    for c in range(NCORES):
        m = core_of == c
        g_e = grp_all[m]
        d_e = dst_all[m]
        s_e = pad_src[m]
        n_e = norm[m]
        # dedup sources within each group
        key = g_e * (NCORES * PADN) + s_e
        ukey, slot_of_edge = np.unique(key, return_inverse=True)
        u_grp = ukey // (NCORES * PADN)
        u_src = ukey % (NCORES * PADN)
        counts = np.bincount(u_grp, minlength=G)
        maxU = max(maxU, counts.max())
        per_core.append((g_e, d_e, s_e, n_e, slot_of_edge, u_grp, u_src,
                         counts))

    C = int((maxU + 127) // 128)
    SLOTS = C * 128
    S16 = SLOTS // 16

    in_maps = []
    for c in range(NCORES):
        g_e, d_e, s_e, n_e, slot_of_edge, u_grp, u_src, counts = per_core[c]
        starts = np.zeros(G + 1, dtype=np.int64)
        np.cumsum(counts, out=starts[1:])
        # slot of each unique (group, src) within its group
        u_slot = np.arange(len(u_grp)) - starts[u_grp]

        # Pads are dummy index 0 (gathered but weighted 0 in S): groups are
        # batched 4-per-gather-instruction, so pads sit interior where the
        # ucode's trailing -1 trim cannot apply.
        idx_l = np.zeros((G, SLOTS), dtype=np.int16)
        idx_l[u_grp, u_slot] = u_src.astype(np.int16)

        S_l = np.zeros((G, SLOTS, 128), dtype=np.float32)
        np.add.at(S_l, (g_e, u_slot[slot_of_edge], d_e), n_e)

        # idx layout: logical slot i -> partition 16*stripe + i%16, col i//16
        idx_rs = idx_l.reshape(G, S16, 16).transpose(2, 0, 1)   # [16, G, S16]
        gidx = np.tile(idx_rs, (8, 1, 1)).astype(np.int16)      # [128, G, S16]

        # S layout: [G, 128(slot%128), C(chunk), 128(dst)]
        S_arr = S_l.reshape(G, C, 128, 128).transpose(0, 2, 1, 3).astype(BF16)

        # self-loop diagonal: D[g, p, p] = dinv[node]^2
        nodes = np.arange(PERCORE) + c * PERCORE
        d2 = np.zeros(PADN, dtype=np.float32)
        d2[:PERCORE] = dinv[nodes] ** 2
        D_arr = np.zeros((G, 128, 128), dtype=np.float32)
        pi = np.arange(128)
        for g in range(G):
            D_arr[g, pi, pi] = d2[g * 128:(g + 1) * 128]
        D_arr = D_arr.astype(BF16)

        xs = x[c * PERCORE:(c + 1) * PERCORE]
        xT = np.zeros((IN_DIM, PADN), dtype=BF16)
        xT[:, :PERCORE] = xs.T.astype(BF16)

        in_maps.append({
            "xT": np.ascontiguousarray(xT),
            "gidx": np.ascontiguousarray(gidx),
            "S": np.ascontiguousarray(S_arr),
            "D": np.ascontiguousarray(D_arr),
        })

    Wc_bf = np.ascontiguousarray(Wc.reshape(L, 2, 128, H).astype(BF16))
    W_in_bf = W_in.astype(BF16)
    biases = np.concatenate([b_in[None, :], bc], axis=0).astype(BF16)
    ln = np.zeros((2 + 2 * L, H), dtype=np.float32)
    ln[0] = g_in
    ln[1] = beta_in
    ln[2:2 + L] = (1.0 - ALPHA) * gc
    ln[2 + L:2 + 2 * L] = (1.0 - ALPHA) * betac
    for m in in_maps:
        m["Wc"] = Wc_bf
        m["W_in"] = W_in_bf
        m["biases"] = biases
        m["ln"] = ln

    return in_maps, C


def _build(cfg, C):
    """Build the Bass program (shared by all 8 cores)."""
    G, PADN, L = cfg.groups, cfg.padn, cfg.layers
    SLOTS = C * 128
    S16 = SLOTS // 16
    f32 = mybir.dt.float32
    i32 = mybir.dt.int32
    bf16 = mybir.dt.bfloat16
    Alu = mybir.AluOpType

    nc = bacc.Bacc("TRN2", target_bir_lowering=False, debug=False,
                   num_devices=NCORES, num_swdge_queues=4)

    xT_in = nc.dram_tensor("xT", [IN_DIM, PADN], bf16, kind="ExternalInput")
    gidx_in = nc.dram_tensor("gidx", [128, G, S16], mybir.dt.int16,
                             kind="ExternalInput")
    S_in = nc.dram_tensor("S", [G, 128, C, 128], bf16, kind="ExternalInput")
    D_in = nc.dram_tensor("D", [G, 128, 128], bf16, kind="ExternalInput")
    Wc_in = nc.dram_tensor("Wc", [L, 2, 128, H], bf16, kind="ExternalInput")
    W_in_in = nc.dram_tensor("W_in", [IN_DIM, H], bf16, kind="ExternalInput")
    biases_in = nc.dram_tensor("biases", [L + 1, H], bf16,
                               kind="ExternalInput")
    ln_in = nc.dram_tensor("ln", [2 + 2 * L, H], f32, kind="ExternalInput")
    out_dram = nc.dram_tensor("out", [PADN, H], f32, kind="ExternalOutput")

    zbounces = [nc.dram_tensor(f"zbounce{l}", [PADN, H], bf16,
                               kind="Internal") for l in range(L)]
    # Shared output -> one-hop peer writes instead of RDH hops
    zfulls = [nc.dram_tensor(f"zfull{l}", [NCORES * PADN, H], bf16,
                             kind="Internal", addr_space="Shared")
              for l in range(L)]

    def bcast128(ap_row):
        return bass.AP(tensor=ap_row.tensor, offset=ap_row.offset,
                       ap=[[0, 128]] + list(ap_row.ap[1:]))

    with tile.TileContext(nc) as tc:
        with (
            tc.tile_pool(name="persist", bufs=1) as pp,
            tc.tile_pool(name="msgs_pool", bufs=4) as msgs_pool,
            tc.tile_pool(name="s_pool", bufs=6) as s_pool,
            tc.tile_pool(name="small", bufs=4) as small,
            tc.tile_pool(name="tiny", bufs=6) as tiny,
            tc.tile_pool(name="psum_a", bufs=3, space="PSUM") as psum_a,
            tc.tile_pool(name="psum_z", bufs=2, space="PSUM") as psum_z,
            tc.tile_pool(name="psum_t", bufs=2, space="PSUM") as psum_t,
        ):
            # ---------- persistent tiles ----------
            xcur = pp.tile([128, G, H], f32)
            h0s = pp.tile([128, G, H], f32)
            z_all = pp.tile([128, G, H], bf16)
            BLK = 1
            NB = G // BLK
            MB = 8
            msgs_all = pp.tile([128, MB, BLK * C, H], bf16)
            y_all = pp.tile([128, G, H], bf16)
            mv_all = pp.tile([128, G, 2], f32)
            rstd_all = pp.tile([128, G], f32)
            eps_sb = pp.tile([128, 1], f32)
            nc.vector.memset(eps_sb[:], EPS)
            gidx_sb = pp.tile([128, G, S16], mybir.dt.int16)
            D_sb = pp.tile([128, G, 128], bf16)
            Wc_sb = pp.tile([128, L * 2, H], bf16)
            W_in_sb = pp.tile([128, H], bf16)
            bias_sb = pp.tile([1, L + 1, H], bf16)
            ones_sb = pp.tile([1, 128], bf16)
            ln_sb = pp.tile([128, 2 + 2 * L, H], f32)
            ident = pp.tile([128, 128], f32)
            xT_sb = pp.tile([128, PADN], bf16)

            nc.sync.dma_start(out=gidx_sb[:], in_=gidx_in.ap())
            for g in range(G):
                nc.sync.dma_start(out=D_sb[:, g, :], in_=D_in.ap()[g])
            for l in range(L):
                for kt in range(2):
                    nc.sync.dma_start(out=Wc_sb[:, l * 2 + kt, :],
                                      in_=Wc_in.ap()[l, kt])
            nc.sync.dma_start(out=W_in_sb[:], in_=W_in_in.ap())
            nc.sync.dma_start(out=bias_sb[:], in_=biases_in.ap()[None])
            nc.vector.memset(ones_sb[:], 1.0)
            for r in range(2 + 2 * L):
                nc.sync.dma_start(out=ln_sb[:, r, :],
                                  in_=bcast128(ln_in.ap()[r:r + 1, :]))
            make_identity(nc, ident[:])
            nc.sync.dma_start(out=xT_sb[:], in_=xT_in.ap())

            def batched_rstd(g0=0, g1=G):
                nc.scalar.activation(
                    out=rstd_all[:, g0:g1], in_=mv_all[:, g0:g1, 1],
                    func=mybir.ActivationFunctionType.Sqrt, bias=eps_sb[:])
                nc.vector.reciprocal(out=rstd_all[:, g0:g1],
                                     in_=rstd_all[:, g0:g1])

            def gelu_stats(g, psum):
                nc.scalar.activation(out=y_all[:, g, :], in_=psum[:],
                                     func=ACT_FN)
                stats = tiny.tile([128, 6], f32, name="bn_st")
                nc.vector.bn_stats(out=stats[:], in_=y_all[:, g, :])
                nc.vector.bn_aggr(out=mv_all[:, g, :], in_=stats[:])

            def ln_blend_z(g, y_ap, mv_ap, rstd_ap, gi, bi, l, first):
                """LN + blend for group g, then z for layer l."""
                t = small.tile([128, H], f32, name="t_ln")
                nc.vector.tensor_scalar_sub(out=t[:], in0=y_ap,
                                            scalar1=mv_ap[0:128, 0:1])
                u = small.tile([128, H], f32, name="u_ln")
                nc.vector.scalar_tensor_tensor(
                    out=u[:], in0=t[:], scalar=rstd_ap, in1=ln_sb[:, gi, :],
                    op0=Alu.mult, op1=Alu.mult)
                if first:
                    nc.vector.tensor_tensor(out=xcur[:, g, :], in0=u[:],
                                            in1=ln_sb[:, bi, :], op=Alu.add)
                    nc.vector.tensor_scalar_mul(out=h0s[:, g, :],
                                                in0=xcur[:, g, :],
                                                scalar1=ALPHA)
                else:
                    v = small.tile([128, H], f32, name="v_ln")
                    nc.vector.tensor_tensor(out=v[:], in0=u[:],
                                            in1=ln_sb[:, bi, :], op=Alu.add)
                    w = small.tile([128, H], f32, name="w_ln")
                    nc.vector.tensor_tensor(out=w[:], in0=v[:],
                                            in1=h0s[:, g, :], op=Alu.add)
                    nc.vector.tensor_tensor(out=xcur[:, g, :],
                                            in0=xcur[:, g, :], in1=w[:],
                                            op=Alu.add)
                if l is not None:
                    # transpose xcur[g], z = xcur @ Wc[l] -> z_all + zbounce
                    tp = psum_t.tile([128, 2, 128], f32, name="tp")
                    xcurT = small.tile([128, 2, 128], bf16, name="xcurT")
                    for kt in range(2):
                        nc.tensor.transpose(
                            out=tp[:, kt, :],
                            in_=xcur[:, g, kt * 128:(kt + 1) * 128],
                            identity=ident[:])
                        nc.scalar.activation(
                            out=xcurT[:, kt, :], in_=tp[:, kt, :],
                            func=mybir.ActivationFunctionType.Copy)
                    zp = psum_z.tile([128, H], f32, name="zp")
                    for kt in range(2):
                        nc.tensor.matmul(
                            out=zp[:], lhsT=xcurT[:, kt, :],
                            rhs=Wc_sb[:, l * 2 + kt, :],
                            start=(kt == 0), stop=(kt == 1))
                    nc.scalar.activation(
                        out=z_all[:, g, :], in_=zp[:],
                        func=mybir.ActivationFunctionType.Copy)
                    nc.sync.dma_start(
                        out=zbounces[l].ap()[g * 128:(g + 1) * 128, :],
                        in_=z_all[:, g, :])
                else:
                    nc.sync.dma_start(
                        out=out_dram.ap()[g * 128:(g + 1) * 128, :],
                        in_=xcur[:, g, :])

            # clear msgs buffers once: -1 pad slots are never written by the
            # gather, and S weights of 0 must multiply finite values
            nc.vector.memset(msgs_all[:], 0.0)

            # ---------- input block ----------
            for g in range(G):
                hp = psum_a.tile([128, H], f32, name="agg")
                nc.tensor.matmul(out=hp[:],
                                 lhsT=xT_sb[:, g * 128:(g + 1) * 128],
                                 rhs=W_in_sb[:], start=True, stop=False)
                nc.tensor.matmul(out=hp[:], lhsT=ones_sb[:],
                                 rhs=bias_sb[:, 0, :], start=False,
                                 stop=True)
                gelu_stats(g, hp)
            batched_rstd()
            for g in range(G):
                ln_blend_z(g, y_all[:, g, :], mv_all[:, g, :],
                           rstd_all[:, g:g + 1], 0, 1, 0, first=True)

            for l in range(L):
                nc.gpsimd.collective_compute(
                    "AllGather", mybir.AluOpType.bypass,
                    replica_groups=[list(range(NCORES))],
                    ins=[zbounces[l].ap()], outs=[zfulls[l].ap()])

                # two half-layers: LN/blend of the first half overlaps the
                # gathers of the second half
                NBH = (NB + 1) // 2
                for half in range(2):
                    b_lo, b_hi = half * NBH, min((half + 1) * NBH, NB)
                    if b_lo >= b_hi:
                        continue
                    for b in range(b_lo, b_hi):
                        buf = (l * NB + b) % MB
                        msgs = msgs_all[:, buf, :, :]
                        nc.gpsimd.dma_gather(
                            msgs, zfulls[l].ap(),
                            gidx_sb[:, b * BLK:(b + 1) * BLK, :],
                            num_idxs=BLK * SLOTS, num_idxs_reg=BLK * SLOTS,
                            elem_size=H, single_packet=False,
                            queue_num=(l * NB + b) % 4)
                        for gsub in range(BLK):
                            g = b * BLK + gsub
                            s_sb = s_pool.tile([128, C, 128], bf16,
                                               name="s_sb")
                            nc.sync.dma_start(out=s_sb[:], in_=S_in.ap()[g])
                            agg = psum_a.tile([128, H], f32, name="agg")
                            # self-loop diagonal first
                            nc.tensor.matmul(out=agg[:], lhsT=D_sb[:, g, :],
                                             rhs=z_all[:, g, :], start=True,
                                             stop=False)
                            for c in range(C):
                                nc.tensor.matmul(
                                    out=agg[:], lhsT=s_sb[:, c, :],
                                    rhs=msgs[:, gsub * C + c, :],
                                    start=False, stop=False)
                            nc.tensor.matmul(
                                out=agg[:], lhsT=ones_sb[:],
                                rhs=bias_sb[:, 1 + l, :],
                                start=False, stop=True)
                            gelu_stats(g, agg)
                    batched_rstd(b_lo * BLK, b_hi * BLK)
                    for g in range(b_lo * BLK, b_hi * BLK):
                        ln_blend_z(g, y_all[:, g, :], mv_all[:, g, :],
                                   rstd_all[:, g:g + 1], 2 + l, 2 + L + l,
                                   l + 1 if l < L - 1 else None, first=False)

    nc.compile()
    return nc


def _get_program(cfg, C):
    key = (cfg, C)
    if key not in _cache:
        _cache[key] = _build(cfg, C)
    return _cache[key]


def run_sharded(inputs, trace=False, cfg=DEFAULT_CFG):
    in_maps, C = _preprocess(cfg, **inputs)
    nc = _get_program(cfg, C)
    res = bass_utils.run_bass_kernel_spmd(
        nc, in_maps, core_ids=list(range(NCORES)), trace=trace)
    out = np.empty((cfg.n, H), dtype=np.float32)
    for c in range(NCORES):
        out[c * cfg.percore:(c + 1) * cfg.percore] = \
            res.results[c]["out"][:cfg.percore]
    return out, res


def kernel(**inputs):
    out, _ = run_sharded(inputs, trace=False)
    return out


# revision 39
# speedup vs baseline: 1.9345x; 1.0259x over previous
"""GCN encoder kernel for Trainium2 (8 NeuronCores).

Strategy (graph/data parallel, per sharding hint):
  - Nodes sharded by destination range across 8 cores (2500 -> padded 2560/core).
  - Host precomputes GCN symmetric normalization and a per-destination-group
    schedule: non-self edges are deduplicated by source per 128-dst group.
    The segment-sum becomes dense matmuls with tiny scatter matrices
    S[src_slot, dst] holding the summed edge norms; self-loops are applied as
    a diagonal matmul against the locally-kept z.
  - Per layer: AllGather z (bf16) across cores, dma_gather the unique source
    rows (2 groups per instruction), TensorE matmuls accumulate messages into
    PSUM per dst group (per-layer bias folded in as a rank-1 matmul), then
    GELU + LayerNorm (rstd batched per layer: one ACT Sqrt + DVE reciprocal,
    so only two ACT table loads per layer) + residual blend.
"""

import sys
from dataclasses import dataclass

sys.path.insert(0, "/opt/trn_rl_repo")

import numpy as np
import ml_dtypes

import concourse.bass as bass
import concourse.tile as tile
from concourse import bacc, mybir
from concourse import bass_utils
from concourse.masks import make_identity

BF16 = ml_dtypes.bfloat16
ALPHA = 0.1
EPS = 1e-5
IN_DIM = 128
H = 256
NCORES = 8
ACT_FN = mybir.ActivationFunctionType.Gelu_apprx_tanh
QUAKE_MAGIC = 0x5F3759DF


@dataclass(frozen=True)
class Cfg:
    n: int = 20000
    layers: int = 6

    @property
    def percore(self):
        return self.n // NCORES

    @property
    def groups(self):
        return (self.percore + 127) // 128

    @property
    def padn(self):
        return self.groups * 128


DEFAULT_CFG = Cfg()
_cache = {}


def _preprocess(cfg, x, edge_index, W_in, b_in, g_in, beta_in, Wc, bc, gc,
                betac):
    """Host-side graph preprocessing -> per-core input maps."""
    N, G, PERCORE, PADN, L = (cfg.n, cfg.groups, cfg.percore, cfg.padn,
                              cfg.layers)
    x = np.asarray(x, dtype=np.float32)
    ei = np.asarray(edge_index).astype(np.int64)
    W_in = np.asarray(W_in, dtype=np.float32)
    b_in = np.asarray(b_in, dtype=np.float32)
    g_in = np.asarray(g_in, dtype=np.float32)
    beta_in = np.asarray(beta_in, dtype=np.float32)
    Wc = np.asarray(Wc, dtype=np.float32)
    bc = np.asarray(bc, dtype=np.float32)
    gc = np.asarray(gc, dtype=np.float32)
    betac = np.asarray(betac, dtype=np.float32)

    loop = np.arange(N, dtype=np.int64)
    col_all = np.concatenate([ei[1], loop])   # dst (for degree)
    deg = np.bincount(col_all, minlength=N).astype(np.float32)
    dinv = np.where(deg > 0, 1.0 / np.sqrt(deg), 0.0).astype(np.float32)

    # non-self edges (self loops handled by the diagonal matmul)
    row = ei[0]
    col = ei[1]
    norm = (dinv[row] * dinv[col]).astype(np.float32)
    pad_src = (row // PERCORE) * PADN + (row % PERCORE)

    core_of = col // PERCORE
    dloc_all = col % PERCORE
    grp_all = dloc_all // 128
    dst_all = dloc_all % 128

    per_core = []
    maxU = 1
    for c in range(NCORES):
        m = core_of == c
        g_e = grp_all[m]
        d_e = dst_all[m]
        s_e = pad_src[m]
        n_e = norm[m]
        # dedup sources within each group
        key = g_e * (NCORES * PADN) + s_e
        ukey, slot_of_edge = np.unique(key, return_inverse=True)
        u_grp = ukey // (NCORES * PADN)
        u_src = ukey % (NCORES * PADN)
        counts = np.bincount(u_grp, minlength=G)
        maxU = max(maxU, counts.max())
        per_core.append((g_e, d_e, s_e, n_e, slot_of_edge, u_grp, u_src,
                         counts))

    C = int((maxU + 127) // 128)
    SLOTS = C * 128
    S16 = SLOTS // 16

    in_maps = []
    for c in range(NCORES):
        g_e, d_e, s_e, n_e, slot_of_edge, u_grp, u_src, counts = per_core[c]
        starts = np.zeros(G + 1, dtype=np.int64)
        np.cumsum(counts, out=starts[1:])
        # slot of each unique (group, src) within its group
        u_slot = np.arange(len(u_grp)) - starts[u_grp]

        # Pads are dummy index 0 (gathered but weighted 0 in S): groups are
        # batched 4-per-gather-instruction, so pads sit interior where the
        # ucode's trailing -1 trim cannot apply.
        idx_l = np.zeros((G, SLOTS), dtype=np.int16)
        idx_l[u_grp, u_slot] = u_src.astype(np.int16)

        S_l = np.zeros((G, SLOTS, 128), dtype=np.float32)
        np.add.at(S_l, (g_e, u_slot[slot_of_edge], d_e), n_e)

        # idx layout: logical slot i -> partition 16*stripe + i%16, col i//16
        idx_rs = idx_l.reshape(G, S16, 16).transpose(2, 0, 1)   # [16, G, S16]
        gidx = np.tile(idx_rs, (8, 1, 1)).astype(np.int16)      # [128, G, S16]

        # S layout: [G, 128(slot%128), C(chunk), 128(dst)]
        S_arr = S_l.reshape(G, C, 128, 128).transpose(0, 2, 1, 3).astype(BF16)

        # self-loop diagonal: D[g, p, p] = dinv[node]^2
        nodes = np.arange(PERCORE) + c * PERCORE
        d2 = np.zeros(PADN, dtype=np.float32)
        d2[:PERCORE] = dinv[nodes] ** 2
        D_arr = np.zeros((G, 128, 128), dtype=np.float32)
        pi = np.arange(128)
        for g in range(G):
            D_arr[g, pi, pi] = d2[g * 128:(g + 1) * 128]
        D_arr = D_arr.astype(BF16)

        xs = x[c * PERCORE:(c + 1) * PERCORE]
        xT = np.zeros((IN_DIM, PADN), dtype=BF16)
        xT[:, :PERCORE] = xs.T.astype(BF16)

        in_maps.append({
            "xT": np.ascontiguousarray(xT),
            "gidx": np.ascontiguousarray(gidx),
            "S": np.ascontiguousarray(S_arr),
            "D": np.ascontiguousarray(D_arr),
        })

    Wc_bf = np.ascontiguousarray(Wc.reshape(L, 2, 128, H).astype(BF16))
    W_in_bf = W_in.astype(BF16)
    biases = np.concatenate([b_in[None, :], bc], axis=0).astype(BF16)
    ln = np.zeros((2 + 2 * L, H), dtype=np.float32)
    ln[0] = g_in
    ln[1] = beta_in
    ln[2:2 + L] = (1.0 - ALPHA) * gc
    ln[2 + L:2 + 2 * L] = (1.0 - ALPHA) * betac
    for m in in_maps:
        m["Wc"] = Wc_bf
        m["W_in"] = W_in_bf
        m["biases"] = biases
        m["ln"] = ln

    return in_maps, C


def _build(cfg, C):
    """Build the Bass program (shared by all 8 cores)."""
    G, PADN, L = cfg.groups, cfg.padn, cfg.layers
    SLOTS = C * 128
    S16 = SLOTS // 16
    f32 = mybir.dt.float32
    i32 = mybir.dt.int32
    bf16 = mybir.dt.bfloat16
    Alu = mybir.AluOpType

    nc = bacc.Bacc("TRN2", target_bir_lowering=False, debug=False,
                   num_devices=NCORES, num_swdge_queues=4)

    xT_in = nc.dram_tensor("xT", [IN_DIM, PADN], bf16, kind="ExternalInput")
    gidx_in = nc.dram_tensor("gidx", [128, G, S16], mybir.dt.int16,
                             kind="ExternalInput")
    S_in = nc.dram_tensor("S", [G, 128, C, 128], bf16, kind="ExternalInput")
    D_in = nc.dram_tensor("D", [G, 128, 128], bf16, kind="ExternalInput")
    Wc_in = nc.dram_tensor("Wc", [L, 2, 128, H], bf16, kind="ExternalInput")
    W_in_in = nc.dram_tensor("W_in", [IN_DIM, H], bf16, kind="ExternalInput")
    biases_in = nc.dram_tensor("biases", [L + 1, H], bf16,
                               kind="ExternalInput")
    ln_in = nc.dram_tensor("ln", [2 + 2 * L, H], f32, kind="ExternalInput")
    out_dram = nc.dram_tensor("out", [PADN, H], f32, kind="ExternalOutput")

    zbounces = [nc.dram_tensor(f"zbounce{l}", [PADN, H], bf16,
                               kind="Internal") for l in range(L)]
    # Shared output -> one-hop peer writes instead of RDH hops
    zfulls = [nc.dram_tensor(f"zfull{l}", [NCORES * PADN, H], bf16,
                             kind="Internal", addr_space="Shared")
              for l in range(L)]

    def bcast128(ap_row):
        return bass.AP(tensor=ap_row.tensor, offset=ap_row.offset,
                       ap=[[0, 128]] + list(ap_row.ap[1:]))

    with tile.TileContext(nc) as tc:
        with (
            tc.tile_pool(name="persist", bufs=1) as pp,
            tc.tile_pool(name="msgs_pool", bufs=4) as msgs_pool,
            tc.tile_pool(name="s_pool", bufs=6) as s_pool,
            tc.tile_pool(name="small", bufs=4) as small,
            tc.tile_pool(name="tiny", bufs=6) as tiny,
            tc.tile_pool(name="psum_a", bufs=4, space="PSUM") as psum_a,
            tc.tile_pool(name="psum_z", bufs=2, space="PSUM") as psum_z,
            tc.tile_pool(name="psum_t", bufs=2, space="PSUM") as psum_t,
        ):
            # ---------- persistent tiles ----------
            xcur = pp.tile([128, G, H], f32)
            h0s = pp.tile([128, G, H], f32)
            z_all = pp.tile([128, G, H], bf16)
            BLK = 1
            NB = G // BLK
            MB = 8
            msgs_all = pp.tile([128, MB, BLK * C, H], bf16)
            y_all = pp.tile([128, G, H], bf16)
            mv_all = pp.tile([128, G, 2], f32)
            rstd_all = pp.tile([128, G], f32)
            eps_sb = pp.tile([128, 1], f32)
            nc.vector.memset(eps_sb[:], EPS)
            gidx_sb = pp.tile([128, G, S16], mybir.dt.int16)
            D_sb = pp.tile([128, G, 128], bf16)
            Wc_sb = pp.tile([128, L * 2, H], bf16)
            W_in_sb = pp.tile([128, H], bf16)
            bias_sb = pp.tile([1, L + 1, H], bf16)
            ones_sb = pp.tile([1, 128], bf16)
            ln_sb = pp.tile([128, 2 + 2 * L, H], f32)
            ident = pp.tile([128, 128], f32)
            xT_sb = pp.tile([128, PADN], bf16)

            nc.sync.dma_start(out=gidx_sb[:], in_=gidx_in.ap())
            for g in range(G):
                nc.sync.dma_start(out=D_sb[:, g, :], in_=D_in.ap()[g])
            for l in range(L):
                for kt in range(2):
                    nc.sync.dma_start(out=Wc_sb[:, l * 2 + kt, :],
                                      in_=Wc_in.ap()[l, kt])
            nc.sync.dma_start(out=W_in_sb[:], in_=W_in_in.ap())
            nc.sync.dma_start(out=bias_sb[:], in_=biases_in.ap()[None])
            nc.vector.memset(ones_sb[:], 1.0)
            for r in range(2 + 2 * L):
                nc.sync.dma_start(out=ln_sb[:, r, :],
                                  in_=bcast128(ln_in.ap()[r:r + 1, :]))
            make_identity(nc, ident[:])
            nc.sync.dma_start(out=xT_sb[:], in_=xT_in.ap())

            def batched_rstd(g0=0, g1=G):
                nc.scalar.activation(
                    out=rstd_all[:, g0:g1], in_=mv_all[:, g0:g1, 1],
                    func=mybir.ActivationFunctionType.Sqrt, bias=eps_sb[:])
                nc.vector.reciprocal(out=rstd_all[:, g0:g1],
                                     in_=rstd_all[:, g0:g1])

            def gelu_stats(g, psum):
                nc.scalar.activation(out=y_all[:, g, :], in_=psum[:],
                                     func=ACT_FN)
                stats = tiny.tile([128, 6], f32, name="bn_st")
                nc.vector.bn_stats(out=stats[:], in_=y_all[:, g, :])
                nc.vector.bn_aggr(out=mv_all[:, g, :], in_=stats[:])

            def ln_blend_z(g, y_ap, mv_ap, rstd_ap, gi, bi, l, first):
                """LN + blend for group g, then z for layer l."""
                t = small.tile([128, H], f32, name="t_ln")
                nc.vector.tensor_scalar_sub(out=t[:], in0=y_ap,
                                            scalar1=mv_ap[0:128, 0:1])
                u = small.tile([128, H], f32, name="u_ln")
                nc.vector.scalar_tensor_tensor(
                    out=u[:], in0=t[:], scalar=rstd_ap, in1=ln_sb[:, gi, :],
                    op0=Alu.mult, op1=Alu.mult)
                if first:
                    nc.vector.tensor_tensor(out=xcur[:, g, :], in0=u[:],
                                            in1=ln_sb[:, bi, :], op=Alu.add)
                    nc.vector.tensor_scalar_mul(out=h0s[:, g, :],
                                                in0=xcur[:, g, :],
                                                scalar1=ALPHA)
                else:
                    v = small.tile([128, H], f32, name="v_ln")
                    nc.vector.tensor_tensor(out=v[:], in0=u[:],
                                            in1=ln_sb[:, bi, :], op=Alu.add)
                    w = small.tile([128, H], f32, name="w_ln")
                    nc.vector.tensor_tensor(out=w[:], in0=v[:],
                                            in1=h0s[:, g, :], op=Alu.add)
                    nc.vector.tensor_tensor(out=xcur[:, g, :],
                                            in0=xcur[:, g, :], in1=w[:],
                                            op=Alu.add)
                if l is not None:
                    # transpose xcur[g], z = xcur @ Wc[l] -> z_all + zbounce
                    tp = psum_t.tile([128, 2, 128], f32, name="tp")
                    xcurT = small.tile([128, 2, 128], bf16, name="xcurT")
                    for kt in range(2):
                        nc.tensor.transpose(
                            out=tp[:, kt, :],
                            in_=xcur[:, g, kt * 128:(kt + 1) * 128],
                            identity=ident[:])
                        nc.scalar.activation(
                            out=xcurT[:, kt, :], in_=tp[:, kt, :],
                            func=mybir.ActivationFunctionType.Copy)
                    zp = psum_z.tile([128, H], f32, name="zp")
                    for kt in range(2):
                        nc.tensor.matmul(
                            out=zp[:], lhsT=xcurT[:, kt, :],
                            rhs=Wc_sb[:, l * 2 + kt, :],
                            start=(kt == 0), stop=(kt == 1))
                    nc.scalar.activation(
                        out=z_all[:, g, :], in_=zp[:],
                        func=mybir.ActivationFunctionType.Copy)
                    nc.sync.dma_start(
                        out=zbounces[l].ap()[g * 128:(g + 1) * 128, :],
                        in_=z_all[:, g, :])
                else:
                    nc.sync.dma_start(
                        out=out_dram.ap()[g * 128:(g + 1) * 128, :],
                        in_=xcur[:, g, :])

            # clear msgs buffers once: -1 pad slots are never written by the
            # gather, and S weights of 0 must multiply finite values
            nc.vector.memset(msgs_all[:], 0.0)

            # ---------- input block ----------
            for g in range(G):
                hp = psum_a.tile([128, H], f32, name="agg")
                nc.tensor.matmul(out=hp[:],
                                 lhsT=xT_sb[:, g * 128:(g + 1) * 128],
                                 rhs=W_in_sb[:], start=True, stop=False)
                nc.tensor.matmul(out=hp[:], lhsT=ones_sb[:],
                                 rhs=bias_sb[:, 0, :], start=False,
                                 stop=True)
                gelu_stats(g, hp)
            batched_rstd()
            for g in range(G):
                ln_blend_z(g, y_all[:, g, :], mv_all[:, g, :],
                           rstd_all[:, g:g + 1], 0, 1, 0, first=True)

            for l in range(L):
                nc.gpsimd.collective_compute(
                    "AllGather", mybir.AluOpType.bypass,
                    replica_groups=[list(range(NCORES))],
                    ins=[zbounces[l].ap()], outs=[zfulls[l].ap()])

                # two half-layers: LN/blend of the first half overlaps the
                # gathers of the second half
                NBH = (NB + 1) // 2
                for half in range(2):
                    b_lo, b_hi = half * NBH, min((half + 1) * NBH, NB)
                    if b_lo >= b_hi:
                        continue
                    for b in range(b_lo, b_hi):
                        buf = (l * NB + b) % MB
                        msgs = msgs_all[:, buf, :, :]
                        nc.gpsimd.dma_gather(
                            msgs, zfulls[l].ap(),
                            gidx_sb[:, b * BLK:(b + 1) * BLK, :],
                            num_idxs=BLK * SLOTS, num_idxs_reg=BLK * SLOTS,
                            elem_size=H, single_packet=False,
                            queue_num=(l * NB + b) % 4)
                        for gsub in range(BLK):
                            g = b * BLK + gsub
                            s_sb = s_pool.tile([128, C, 128], bf16,
                                               name="s_sb")
                            nc.sync.dma_start(out=s_sb[:], in_=S_in.ap()[g])
                            agg = psum_a.tile([128, H], f32, name="agg")
                            # self-loop diagonal first
                            nc.tensor.matmul(out=agg[:], lhsT=D_sb[:, g, :],
                                             rhs=z_all[:, g, :], start=True,
                                             stop=False)
                            for c in range(C):
                                nc.tensor.matmul(
                                    out=agg[:], lhsT=s_sb[:, c, :],
                                    rhs=msgs[:, gsub * C + c, :],
                                    start=False, stop=False)
                            nc.tensor.matmul(
                                out=agg[:], lhsT=ones_sb[:],
                                rhs=bias_sb[:, 1 + l, :],
                                start=False, stop=True)
                            gelu_stats(g, agg)
                    batched_rstd(b_lo * BLK, b_hi * BLK)
                    for g in range(b_lo * BLK, b_hi * BLK):
                        ln_blend_z(g, y_all[:, g, :], mv_all[:, g, :],
                                   rstd_all[:, g:g + 1], 2 + l, 2 + L + l,
                                   l + 1 if l < L - 1 else None, first=False)

    nc.compile()
    return nc


def _get_program(cfg, C):
    key = (cfg, C)
    if key not in _cache:
        _cache[key] = _build(cfg, C)
    return _cache[key]


def run_sharded(inputs, trace=False, cfg=DEFAULT_CFG):
    in_maps, C = _preprocess(cfg, **inputs)
    nc = _get_program(cfg, C)
    res = bass_utils.run_bass_kernel_spmd(
        nc, in_maps, core_ids=list(range(NCORES)), trace=trace)
    out = np.empty((cfg.n, H), dtype=np.float32)
    for c in range(NCORES):
        out[c * cfg.percore:(c + 1) * cfg.percore] = \
            res.results[c]["out"][:cfg.percore]
    return out, res


def kernel(**inputs):
    out, _ = run_sharded(inputs, trace=False)
    return out
